# revision 18
# baseline (speedup 1.0000x reference)
import sys
sys.path.insert(0, '/opt/trn_rl_repo')

import numpy as np
import ml_dtypes

T, B, C, H, W = 4, 32, 64, 56, 56
NCORES = 8
BL = B // NCORES
NIMG = T * BL
HP = W + 2
PP = HP * HP
PIX = H * W
NCH = 7
CHW = 8 * W
NPAIR = 8
EPS = 1e-5
NG = float((T * B) * PIX)
QL = 14 * W
NQ = 4
E4 = ml_dtypes.float8_e4m3
E5 = ml_dtypes.float8_e5m2

_prog_cache = {}
NO_CC = False
TRACE = False
LAST_RES = None
LAST_NAMES = None


def _build(alpha1, alpha2):
    import concourse.mybir as mybir
    import concourse.tile as tile
    from concourse import bacc

    F32 = mybir.dt.float32
    F16 = mybir.dt.float16
    F8E4 = mybir.dt.float8e4
    F8E5 = mybir.dt.float8e5
    AO = mybir.AluOpType
    AF = mybir.ActivationFunctionType
    AX = mybir.AxisListType
    PM = mybir.MatmulPerfMode

    V1 = float(np.frombuffer(np.array([0x2C3C], np.uint16).tobytes(),
                             dtype=np.float16)[0])
    V2 = float(np.frombuffer(np.array([0x0C1C], np.uint16).tobytes(),
                             dtype=np.float16)[0])

    nc = bacc.Bacc(None, target_bir_lowering=False)
    names = {}

    with tile.TileContext(nc) as tc:
        with tc.tile_pool(name="dram", bufs=1, space="DRAM") as dram:
            xta = dram.tile([NIMG, 2, 64, PP], F16, kind="ExternalInput")
            xtb = dram.tile([NIMG, 2, 64, PP], F16, kind="ExternalInput")
            xtc = dram.tile([NIMG, 2, 64, PP], F16, kind="ExternalInput")
            xin = dram.tile([NIMG, 64, PIX], F32, kind="ExternalInput")
            w1a = dram.tile([128, 9 * 64], F16, kind="ExternalInput")
            w1p = dram.tile([128, 3 * 64], F16, kind="ExternalInput")
            w1s = dram.tile([128, 3 * 64], F16, kind="ExternalInput")
            w1q = dram.tile([128, 64], F16, kind="ExternalInput")
            w2d1 = dram.tile([128, 2, 9, 128], F8E4, kind="ExternalInput")
            w2d2 = dram.tile([128, 2, 9, 128], F8E4, kind="ExternalInput")
            cpar = dram.tile([128, 8], F32, kind="ExternalInput")
            ident = dram.tile([128, 128], F32, kind="ExternalInput")
            outp = dram.tile([NIMG, 64, PIX], F8E4, kind="ExternalOutput")
            names.update(xta=xta.name, xtb=xtb.name, xtc=xtc.name,
                         xin=xin.name,
                         w1a=w1a.name, w1p=w1p.name, w1s=w1s.name,
                         w1q=w1q.name,
                         w2d1=w2d1.name, w2d2=w2d2.name,
                         cpar=cpar.name, ident=ident.name,
                         outp=outp.name)

            with tc.tile_pool(name="dramw", bufs=1, space="DRAM") as dramw, \
                 tc.tile_pool(name="wsb", bufs=1) as wsb, \
                 tc.tile_pool(name="ys", bufs=8) as yspool, \
                 tc.tile_pool(name="plane", bufs=4) as plpool, \
                 tc.tile_pool(name="plb", bufs=2) as plbpool, \
                 tc.tile_pool(name="hfp", bufs=2) as hf, \
                 tc.tile_pool(name="tiny", bufs=8) as tiny, \
                 tc.tile_pool(name="ps", bufs=7, space="PSUM") as ps:

                w1as = wsb.tile([128, 9 * 64], F16, tag="w1a")
                nc.scalar.dma_start(w1as[:], w1a[:])
                w1ps = wsb.tile([128, 3 * 64], F16, tag="w1p")
                nc.scalar.dma_start(w1ps[:], w1p[:])
                w1ss = wsb.tile([128, 3 * 64], F16, tag="w1s")
                nc.scalar.dma_start(w1ss[:], w1s[:])
                w1qs = wsb.tile([128, 64], F16, tag="w1q")
                nc.scalar.dma_start(w1qs[:], w1q[:])
                w2d1s = wsb.tile([128, 2, 9, 128], F8E4, tag="w2d1")
                nc.scalar.dma_start(w2d1s[:], w2d1[:])
                w2d2s = wsb.tile([128, 2, 9, 128], F8E4, tag="w2d2")
                nc.scalar.dma_start(w2d2s[:], w2d2[:])
                cpars = wsb.tile([128, 8], F32, tag="cpar")
                nc.scalar.dma_start(cpars[:], cpar[:])
                idents = wsb.tile([128, 128], F32, tag="ident")
                nc.scalar.dma_start(idents[:], ident[:])
                sums1 = wsb.tile([128, 56], F32, tag="sums1")
                sums1q = wsb.tile([128, 56], F32, tag="sums1q")
                sums2 = wsb.tile([128, 56], F32, tag="sums2")
                sums2q = wsb.tile([128, 56], F32, tag="sums2q")
                scrv = wsb.tile([128, CHW], F32, tag="scrv")
                scra = wsb.tile([128, CHW], F32, tag="scra")
                sqwarm = tiny.tile([128, 1], F32, tag="t1")
                nc.vector.memset(sqwarm[:], 1.0)
                sqw2 = tiny.tile([128, 1], F32, tag="t1")
                nc.scalar.activation(sqw2[:], sqwarm[:], AF.Sqrt)

                def evac(pts, dst, sums_t, sumsq_t, col, on_act=False):
                    nc.scalar.activation(dst, pts[:], AF.Copy,
                                         accum_out=sums_t[:, col:col + 1])
                    if on_act:
                        nc.scalar.activation(
                            scra[:], dst, AF.Square,
                            accum_out=sumsq_t[:, col:col + 1])
                    else:
                        nc.vector.scalar_tensor_tensor(
                            scrv[:], dst, 1.0, dst, AO.bypass, AO.mult,
                            accum_out=sumsq_t[:, col:col + 1])

                def conv1_pair(plA, plB, pbA, pbB, pcA, pcB, dst_strip,
                               pcol):
                    plAr = plA.rearrange("p (r w) -> p r w", w=HP)
                    plBr = plB.rearrange("p (r w) -> p r w", w=HP)
                    pbAr = pbA.rearrange("p (r w) -> p r w", w=HP)
                    pbBr = pbB.rearrange("p (r w) -> p r w", w=HP)
                    pcAr = pcA.rearrange("p (r w) -> p r w", w=HP)
                    pcBr = pcB.rearrange("p (r w) -> p r w", w=HP)
                    for wave in (range(0, 4), range(4, 7)):
                        pts = {}
                        for cth in wave:
                            pts[cth] = ps.tile([128, CHW], F32, tag="ps",
                                               bufs=7, name=f"psum{cth}")
                        def hi_tap(a, start, stop):
                            di, dj = a // 3, a % 3
                            for cth in wave:
                                r0 = 8 * cth + di
                                for j, plr in enumerate((plAr, plBr)):
                                    out = pts[cth][64 * j:64 * (j + 1), :] \
                                        .rearrange("p (r w) -> p r w", r=8)
                                    nc.tensor.matmul(
                                        out, w1as[:, a * 64:(a + 1) * 64],
                                        plr[:, r0:r0 + 8, dj:dj + W],
                                        start=start, stop=stop,
                                        tile_position=(0, 64 * j),
                                        skip_group_check=True)

                        hi_tap(0, True, False)
                        for a in range(1, 5):
                            hi_tap(a, False, False)
                        for di in range(3):
                            for cth in wave:
                                r0 = 8 * cth + di
                                for j, pbr in enumerate((pbAr, pbBr)):
                                    out = pts[cth][64 * j:64 * (j + 1), :] \
                                        .rearrange("p (r w) -> p r w", r=8)
                                    nc.tensor.matmul(
                                        out, w1ps[:, di * 64:(di + 1) * 64],
                                        pbr[:, r0:r0 + 8, 0:W],
                                        start=False, stop=False,
                                        tile_position=(0, 64 * j),
                                        skip_group_check=True)
                        for cth in wave:
                            r0 = 8 * cth
                            for j, pcr in enumerate((pcAr, pcBr)):
                                out = pts[cth][64 * j:64 * (j + 1), :] \
                                    .rearrange("p (r w) -> p r w", r=8)
                                nc.tensor.matmul(
                                    out, w1qs[:, 0:64],
                                    pcr[:, r0:r0 + 8, 0:W],
                                    start=False, stop=False,
                                    tile_position=(0, 64 * j),
                                    skip_group_check=True)
                        for cth in wave:
                            r0 = 8 * cth + 2
                            for j, plr in enumerate((plAr, plBr)):
                                out = pts[cth][64 * j:64 * (j + 1), :] \
                                    .rearrange("p (r w) -> p r w", r=8)
                                nc.tensor.matmul(
                                    out, w1ss[0:64, 2 * 64:3 * 64],
                                    plr[0:64, r0:r0 + 8, 2:2 + W],
                                    start=False, stop=False,
                                    tile_position=(0, 64 * j),
                                    skip_group_check=True)
                        for a in range(5, 9):
                            hi_tap(a, False, a == 8)
                        for cth in wave:
                            evac(pts[cth],
                                 dst_strip[:, CHW * cth:CHW * (cth + 1)],
                                 sums1, sums1q, pcol * 7 + cth,
                                 on_act=(cth % 2 == 1))

                def conv2_pair(sp1, sp2, dst_strip, pcol):
                    p14 = sp1.rearrange("p (r w) k -> p k r w", w=HP)
                    p24 = sp2.rearrange("p (r w) k -> p k r w", w=HP)
                    for wave in (range(0, 4), range(4, 7)):
                        pts = {}
                        for cth in wave:
                            pts[cth] = ps.tile([128, CHW], F32, tag="ps",
                                               bufs=7, name=f"psum{cth}")
                        for pli, (pl4, wd) in enumerate(
                                ((p14, w2d1s), (p24, w2d2s))):
                            for a in range(9):
                                di, dj = a // 3, a % 3
                                for cth in wave:
                                    r0 = 8 * cth + di
                                    out = pts[cth][:].rearrange(
                                        "p (r w) -> p r w", r=8)
                                    nc.tensor.matmul(
                                        out, wd[:, :, a, :],
                                        pl4[:, :, r0:r0 + 8, dj:dj + W],
                                        start=(pli == 0 and a == 0),
                                        stop=(pli == 1 and a == 8),
                                        perf_mode=PM.DoubleRow,
                                        skip_group_check=True)
                        for cth in wave:
                            evac(pts[cth],
                                 dst_strip[:, CHW * cth:CHW * (cth + 1)],
                                 sums2, sums2q, pcol * 7 + cth,
                                 on_act=(cth % 2 == 1))

                y1s = []
                for p in range(NPAIR):
                    tt_, bp = p // 2, p % 2
                    iA = tt_ * 4 + bp * 2
                    tas, pbs, pcs = [], [], []
                    for j in range(2):
                        i = iA + j
                        ta = plpool.tile([128, PP], F16, tag="pl")
                        tar = ta.rearrange("p (r w) -> p r w", w=HP)
                        xtr = xta[i].rearrange("a c (r w) -> a c r w", w=HP)
                        nc.sync.dma_start(tar[:, 0:29, :],
                                          xtr[:, :, 0:29, :])
                        nc.gpsimd.dma_start(tar[:, 29:HP, :],
                                            xtr[:, :, 29:HP, :])
                        tas.append(ta)
                        pb = plbpool.tile([128, PP], F16, tag="plb",
                                          bufs=3)
                        nc.scalar.dma_start(pb[:], xtb[i])
                        pbs.append(pb)
                        pc = plbpool.tile([128, PP], F16, tag="plb",
                                          bufs=3)
                        nc.sync.dma_start(pc[:], xtc[i])
                        pcs.append(pc)
                    strip = yspool.tile([128, PIX], F32, tag=f"ys{p}",
                                        bufs=1)
                    y1s.append(strip)
                    conv1_pair(tas[0], tas[1], pbs[0], pbs[1], pcs[0],
                               pcs[1], strip, p)

                cc1i = dramw.tile([128, 2], F32)
                cc1o = dramw.tile([128, 2], F32, addr_space="Shared")
                acc1 = tiny.tile([128, 2], F32, tag="acc")
                nc.vector.tensor_reduce(acc1[:, 0:1], sums1[:], AX.X, AO.add)
                nc.vector.tensor_reduce(acc1[:, 1:2], sums1q[:], AX.X, AO.add)
                nc.sync.dma_start(cc1i[:], acc1[:])
                if NO_CC:
                    nc.sync.dma_start(cc1o[:], cc1i[:])
                else:
                    nc.gpsimd.collective_compute(
                        "AllReduce", AO.add, ins=[cc1i[:]], outs=[cc1o[:]],
                        replica_groups=[list(range(NCORES))])
                g1 = tiny.tile([128, 2], F32, tag="acc")
                nc.sync.dma_start(g1[:], cc1o[:])

                def stats_block(g, gamma, beta, rga, rgam, alpha):
                    totp = ps.tile([128, CHW], F32, tag="pstot", bufs=1,
                                   name="pstot")
                    nc.tensor.matmul(totp[:, 0:2], idents[:], g[:],
                                     start=True, stop=True,
                                     skip_group_check=True)
                    tot = totp[:, 0:2]
                    mm = tiny.tile([128, 2], F32, tag="acc")
                    nc.vector.tensor_scalar(mm[:], tot, 1.0 / NG, None,
                                            AO.mult)
                    mean = mm[:, 0:1]
                    m2 = tiny.tile([128, 1], F32, tag="t1")
                    nc.vector.tensor_tensor(m2[:], mean, mean, AO.mult)
                    var = tiny.tile([128, 1], F32, tag="t1")
                    nc.vector.tensor_tensor(var[:], mm[:, 1:2], m2[:],
                                            AO.subtract)
                    epst = tiny.tile([128, 1], F32, tag="t1")
                    nc.vector.memset(epst[:], EPS)
                    std = tiny.tile([128, 1], F32, tag="t1")
                    nc.scalar.activation(std[:], var[:], AF.Sqrt, bias=epst[:])
                    rstd = tiny.tile([128, 1], F32, tag="t1")
                    nc.vector.reciprocal(rstd[:], std[:])
                    rscv = tiny.tile([128, 1], F32, tag="t1")
                    nc.vector.tensor_tensor(rscv[:], std[:], rgam, AO.mult)
                    sc = tiny.tile([128, 1], F32, tag="t1")
                    nc.vector.tensor_tensor(sc[:], gamma, rstd[:], AO.mult)
                    nmsc = tiny.tile([128, 1], F32, tag="t1")
                    nc.vector.scalar_tensor_tensor(nmsc[:], mean, -1.0, sc[:],
                                                   AO.mult, AO.mult)
                    bi = tiny.tile([128, 1], F32, tag="t1")
                    nc.vector.tensor_tensor(bi[:], beta, nmsc[:], AO.add)
                    stdrg = tiny.tile([128, 1], F32, tag="t1")
                    nc.vector.tensor_tensor(stdrg[:], std[:], rga, AO.mult)
                    nbst = tiny.tile([128, 1], F32, tag="t1")
                    nc.vector.scalar_tensor_tensor(nbst[:], bi[:], -alpha,
                                                   stdrg[:], AO.mult, AO.mult)
                    th = tiny.tile([128, 1], F32, tag="t1")
                    nc.vector.tensor_tensor(th[:], stdrg[:], nbst[:], AO.add)
                    bstd = tiny.tile([128, 1], F32, tag="t1")
                    nc.vector.tensor_tensor(bstd[:], bi[:], std[:], AO.mult)
                    gamv = tiny.tile([128, 1], F32, tag="t1")
                    nc.vector.tensor_tensor(gamv[:], bstd[:], rgam, AO.mult)
                    gmw = tiny.tile([128, 1], F32, tag="t1")
                    nc.vector.tensor_scalar(gmw[:], gamv[:], 1.0 - alpha, None,
                                            AO.mult)
                    return th, gamv, rscv, gmw

                th1, gm1, _rsc1, gmw1 = stats_block(
                    g1, cpars[:, 0:1], cpars[:, 1:2], cpars[:, 4:5],
                    cpars[:, 6:7], alpha1)

                y2s = [None] * NPAIR
                for bp in range(2):
                    Pprev = [None] * NQ
                    for t in range(1, 5):
                        p = (t - 1) * 2 + bp
                        sp1 = plpool.tile([128, PP, 2], F8E5, tag="pl")
                        sp2 = plpool.tile([128, PP, 2], F8E5, tag="pl")
                        w1v = sp1.rearrange("p (r w) k -> p r w k", w=HP)
                        w2v = sp2.rearrange("p (r w) k -> p r w k", w=HP)
                        for sp4 in (w1v, w2v):
                            nc.gpsimd.memset(sp4[:, 0:1, :, :], 0.0)
                            nc.gpsimd.memset(sp4[:, HP - 1:HP, :, :], 0.0)
                            nc.gpsimd.memset(sp4[:, 1:HP - 1, 0:1, :], 0.0)
                            nc.gpsimd.memset(sp4[:, 1:HP - 1,
                                                 HP - 1:HP, :], 0.0)
                        for hq in range(NQ):
                            off = QL * hq
                            ysl = y1s[p][:, off:off + QL]
                            if t == 1:
                                qa = ysl
                            else:
                                q = hf.tile([128, QL], F32, tag="q2", bufs=4)
                                if hq % 2 == 0:
                                    nc.vector.affine_then_add(
                                        q[:], Pprev[hq][:], ysl, 1.0, 0.0)
                                else:
                                    nc.gpsimd.tensor_tensor(
                                        q[:], ysl, Pprev[hq][:], AO.add)
                                qa = q[:]
                            qa3 = qa.rearrange("p (r w) -> p r w", w=W)
                            r0, r1_ = 1 + 14 * hq, 15 + 14 * hq
                            d1 = w1v[:, r0:r1_, 1:1 + W, :].bitcast(F16)
                            d2 = w2v[:, r0:r1_, 1:1 + W, :].bitcast(F16)
                            seng = nc.vector if hq % 2 == 0 else nc.gpsimd
                            nc.vector.tensor_scalar(d1, qa3, th1[:],
                                                    V1, AO.is_ge, AO.mult)
                            seng.tensor_scalar(d2, qa3, th1[:],
                                               V2, AO.is_ge, AO.mult)
                            if t < 4:
                                wv = hf.tile([128, QL], F32, tag="wv",
                                             bufs=2)
                                nc.scalar.activation(wv[:], qa, AF.Identity,
                                                     bias=gmw1[:],
                                                     scale=1.0 - alpha1)
                                Pn = hf.tile([128, QL], F32, tag="pp",
                                             bufs=4)
                                nc.vector.scalar_tensor_tensor(
                                    Pn[:], qa, th1[:], wv[:],
                                    AO.is_lt, AO.mult)
                                Pprev[hq] = Pn
                        strip2 = yspool.tile([128, PIX], F32, tag=f"ys{p}",
                                             bufs=1)
                        y2s[p] = strip2
                        conv2_pair(sp1, sp2, strip2, p)

                cc2i = dramw.tile([128, 2], F32)
                cc2o = dramw.tile([128, 2], F32, addr_space="Shared")
                acc2 = tiny.tile([128, 2], F32, tag="acc")
                nc.vector.tensor_reduce(acc2[:, 0:1], sums2[:], AX.X, AO.add)
                nc.vector.tensor_reduce(acc2[:, 1:2], sums2q[:], AX.X, AO.add)
                nc.sync.dma_start(cc2i[:], acc2[:])
                if NO_CC:
                    nc.sync.dma_start(cc2o[:], cc2i[:])
                else:
                    nc.gpsimd.collective_compute(
                        "AllReduce", AO.add, ins=[cc2i[:]], outs=[cc2o[:]],
                        replica_groups=[list(range(NCORES))])
                g2 = tiny.tile([128, 2], F32, tag="acc")
                nc.sync.dma_start(g2[:], cc2o[:])
                th2, gm2, rsc2, gmw2 = stats_block(
                    g2, cpars[:, 2:3], cpars[:, 3:4], cpars[:, 5:6],
                    cpars[:, 7:8], alpha2)
                nth2 = tiny.tile([128, 1], F32, tag="t1")
                nc.vector.tensor_scalar(nth2[:], th2[:], -1.0, None, AO.mult)
                gw2 = tiny.tile([128, 1], F32, tag="t1")
                nc.vector.scalar_tensor_tensor(gw2[:], th2[:], 1.0 - alpha2,
                                               gmw2[:], AO.mult, AO.add)

                PYprev = {0: [None] * NQ, 1: [None] * NQ}
                iters = [(t, bp, hq) for t in range(1, 5)
                         for bp in range(2) for hq in range(NQ)]
                KPF = 3
                xstiles = {}

                def issue_load(idx):
                    t, bp, hq = iters[idx]
                    iA = (t - 1) * 4 + bp * 2
                    off = QL * hq
                    xs = hf.tile([128, QL], F32, tag="xs", bufs=5)
                    nc.sync.dma_start(xs[:],
                                      xin[iA:iA + 2, :, off:off + QL])
                    xstiles[idx] = xs

                wvtiles = {}
                ottiles = {}

                def flush_tail(idx):
                    t, bp, hq = iters[idx]
                    iA = (t - 1) * 4 + bp * 2
                    off = QL * hq
                    xs = xstiles.pop(idx)
                    if t < 4:
                        ptag = ("pp", 4) if bp == 0 else ("q2", 4)
                        Pn = hf.tile([128, QL], F32, tag=ptag[0],
                                     bufs=ptag[1])
                        nc.vector.scalar_tensor_tensor(
                            Pn[:], xs[:], 0.0, wvtiles.pop(idx)[:],
                            AO.is_lt, AO.mult)
                        p_next = t * 2 + bp
                        nc.gpsimd.tensor_tensor(
                            Pn[:], Pn[:], y2s[p_next][:, off:off + QL],
                            AO.add)
                        PYprev[bp][hq] = Pn
                    nc.gpsimd.dma_start(outp[iA:iA + 2, :, off:off + QL],
                                        ottiles.pop(idx)[:])

                for idx in range(KPF):
                    issue_load(idx)
                for idx, (t, bp, hq) in enumerate(iters):
                    if idx + KPF < len(iters):
                        issue_load(idx + KPF)
                    p = (t - 1) * 2 + bp
                    off = QL * hq
                    xs = xstiles[idx]
                    py = y2s[p][:, off:off + QL] if t == 1 \
                        else PYprev[bp][hq][:]
                    nc.vector.affine_then_add(xs[:], xs[:], py,
                                              rsc2[:], nth2[:])
                    if t < 4:
                        wv2 = hf.tile([128, QL], F32, tag="wv", bufs=2)
                        nc.scalar.activation(wv2[:], xs[:], AF.Identity,
                                             bias=gw2[:],
                                             scale=1.0 - alpha2)
                        wvtiles[idx] = wv2
                    ot = hf.tile([128, QL], F8E4, tag="ot", bufs=2)
                    nc.scalar.activation(ot[:], xs[:], AF.Sign)
                    ottiles[idx] = ot
                    if idx > 0:
                        flush_tail(idx - 1)
                flush_tail(len(iters) - 1)

    nc.compile()
    return nc, names


def _sigmoid(x):
    return 1.0 / (1.0 + np.exp(-float(x)))


def prepare(x, conv1_w, bn1_gamma, bn1_beta, lif1_w, conv2_w, bn2_gamma,
            bn2_beta, lif2_w):
    x = np.ascontiguousarray(np.asarray(x, np.float32))
    conv1_w = np.asarray(conv1_w, np.float32)
    conv2_w = np.asarray(conv2_w, np.float32)

    a1 = _sigmoid(np.asarray(lif1_w).reshape(-1)[0])
    a2 = _sigmoid(np.asarray(lif2_w).reshape(-1)[0])

    key = (round(a1, 12), round(a2, 12))
    if key not in _prog_cache:
        _prog_cache[key] = _build(a1, a2)
    nc, names = _prog_cache[key]

    xh = x.astype(np.float16)
    xl = (x - xh.astype(np.float32)).astype(np.float16)
    xpad = np.zeros((T, B, C, 2, HP, HP), np.float16)
    xpad[:, :, :, 0, 1:57, 1:57] = xh
    xpad[:, :, :, 1, 1:57, 1:57] = xl
    xpad = np.ascontiguousarray(xpad.transpose(0, 1, 3, 2, 4, 5))

    xhf = xpad[:, :, 0].reshape(T, B, C, PP)
    xbs = np.zeros((T, B, 2, C, PP), np.float16)
    xbs[:, :, 0] = xhf
    xbs[:, :, 1, :, :PP - 1] = xhf[:, :, :, 1:]
    xcs = np.zeros((T, B, 2, C, PP), np.float16)
    xcs[:, :, 0, :, :PP - 2] = xhf[:, :, :, 2:]
    xcs[:, :, 1, :, :PP - HP - 2] = xhf[:, :, :, HP + 2:]

    w1h = conv1_w.astype(np.float16)
    w1l = (conv1_w - w1h.astype(np.float32)).astype(np.float16)

    def tapstack(wtop, wbot):
        out = np.zeros((128, 9 * 64), np.float16)
        for a in range(9):
            di, dj = a // 3, a % 3
            out[0:64, a * 64:(a + 1) * 64] = wtop[:, :, di, dj].T
            out[64:128, a * 64:(a + 1) * 64] = wbot[:, :, di, dj].T
        return out

    w1a_np = tapstack(w1h, w1h)
    w1q_np = np.zeros((128, 64), np.float16)
    w1q_np[0:64, :] = w1l[:, :, 0, 2].T
    w1q_np[64:128, :] = w1l[:, :, 1, 2].T
    w1p_np = np.zeros((128, 3 * 64), np.float16)
    w1s_np = np.zeros((128, 3 * 64), np.float16)
    for di in range(3):
        w1p_np[0:64, di * 64:(di + 1) * 64] = w1l[:, :, di, 0].T
        w1p_np[64:128, di * 64:(di + 1) * 64] = w1l[:, :, di, 1].T
        w1s_np[0:64, di * 64:(di + 1) * 64] = w1l[:, :, di, 2].T

    w0 = conv2_w.astype(E4)
    r1 = conv2_w - w0.astype(np.float32)
    w1t = (r1 * 16).astype(E4)
    r2 = r1 - w1t.astype(np.float32) / 16
    w2t = (r2 * 256).astype(E4)
    r3 = r2 - w2t.astype(np.float32) / 256
    w3t = (r3 * 4096).astype(E4)
    w2d1_np = np.zeros((128, 2, 9, 128), E4)
    w2d2_np = np.zeros((128, 2, 9, 128), E4)
    for a in range(9):
        di, dj = a // 3, a % 3
        for blk in range(2):
            sl = slice(64 * blk, 64 * blk + 64)
            w2d1_np[sl, 0, a, sl] = w0[:, :, di, dj].T
            w2d1_np[sl, 1, a, sl] = w1t[:, :, di, dj].T
            w2d2_np[sl, 0, a, sl] = w2t[:, :, di, dj].T
            w2d2_np[sl, 1, a, sl] = w3t[:, :, di, dj].T

    def dup(v):
        v = np.asarray(v, np.float32).reshape(64)
        return np.concatenate([v, v])

    cpar_np = np.zeros((128, 8), np.float32)
    cpar_np[:, 0] = dup(bn1_gamma)
    cpar_np[:, 1] = dup(bn1_beta)
    cpar_np[:, 2] = dup(bn2_gamma)
    cpar_np[:, 3] = dup(bn2_beta)
    cpar_np[:, 4] = 1.0 / (a1 * dup(bn1_gamma))
    cpar_np[:, 5] = 1.0 / (a2 * dup(bn2_gamma))
    cpar_np[:, 6] = 1.0 / dup(bn1_gamma)
    cpar_np[:, 7] = 1.0 / dup(bn2_gamma)

    kk, mm_ = np.meshgrid(np.arange(128), np.arange(128), indexing='ij')
    ident_np = (kk % 64 == mm_ % 64).astype(np.float32)

    in_maps = []
    for k in range(NCORES):
        xta_np = np.ascontiguousarray(
            xpad[:, 4 * k:4 * k + 4].reshape(NIMG, 2, 64, PP))
        xtb_np = np.ascontiguousarray(
            xbs[:, 4 * k:4 * k + 4].reshape(NIMG, 2, 64, PP))
        xtc_np = np.ascontiguousarray(
            xcs[:, 4 * k:4 * k + 4].reshape(NIMG, 2, 64, PP))
        xin_np = np.ascontiguousarray(
            x[:, 4 * k:4 * k + 4].reshape(NIMG, 64, PIX))
        in_maps.append({
            names['xta']: xta_np,
            names['xtb']: xtb_np,
            names['xtc']: xtc_np,
            names['w1q']: w1q_np,
            names['xin']: xin_np,
            names['w1a']: w1a_np,
            names['w1p']: w1p_np,
            names['w1s']: w1s_np,
            names['w2d1']: w2d1_np,
            names['w2d2']: w2d2_np,
            names['cpar']: cpar_np,
            names['ident']: ident_np,
        })

    return nc, names, in_maps


def kernel(**inputs):
    from concourse.bass_utils import run_bass_kernel_spmd
    nc, names, in_maps = prepare(**inputs)
    res = run_bass_kernel_spmd(nc, in_maps, core_ids=list(range(NCORES)))
    global LAST_RES, LAST_NAMES
    LAST_RES, LAST_NAMES = res, names
    out = np.empty((T, B, C, H, W), np.float32)
    for k in range(NCORES):
        o = res.results[k][names['outp']].astype(np.float32)
        o = (o >= -0.5).astype(np.float32)
        out[:, 4 * k:4 * k + 4] = o.reshape(T, BL, C, H, W)
    return out


if __name__ == "__main__":
    rng = np.random.default_rng(0)
    xs = rng.standard_normal((T, B, C, H, W)).astype(np.float32)
    w1 = (rng.standard_normal((64, 64, 3, 3)) * 0.05).astype(np.float32)
    w2 = (rng.standard_normal((64, 64, 3, 3)) * 0.05).astype(np.float32)
    o = kernel(x=xs, conv1_w=w1, bn1_gamma=np.ones(64, np.float32),
               bn1_beta=np.zeros(64, np.float32),
               lif1_w=np.zeros(1, np.float32), conv2_w=w2,
               bn2_gamma=np.ones(64, np.float32),
               bn2_beta=np.zeros(64, np.float32),
               lif2_w=np.zeros(1, np.float32))
    print("ran:", o.shape, float(o.mean()))


# revision 19
# speedup vs baseline: 1.1742x; 1.1742x over previous
import sys
sys.path.insert(0, '/opt/trn_rl_repo')

import numpy as np
import ml_dtypes

T, B, C, H, W = 4, 32, 64, 56, 56
NCORES = 8
BL = B // NCORES
NIMG = T * BL
HP = W + 2
PP = HP * HP
PIX = H * W
NCH = 7
CHW = 8 * W
NPAIR = 8
EPS = 1e-5
NG = float((T * B) * PIX)
QL = 14 * W
NQ = 4
E4 = ml_dtypes.float8_e4m3
E5 = ml_dtypes.float8_e5m2

_prog_cache = {}
NO_CC = False
TRACE = False
LAST_RES = None
LAST_NAMES = None


def _build(alpha1, alpha2):
    import concourse.mybir as mybir
    import concourse.tile as tile
    from concourse import bacc

    F32 = mybir.dt.float32
    F16 = mybir.dt.float16
    F8E4 = mybir.dt.float8e4
    F8E5 = mybir.dt.float8e5
    AO = mybir.AluOpType
    AF = mybir.ActivationFunctionType
    AX = mybir.AxisListType
    PM = mybir.MatmulPerfMode

    V1 = float(np.frombuffer(np.array([0x2C3C], np.uint16).tobytes(),
                             dtype=np.float16)[0])
    V2 = float(np.frombuffer(np.array([0x0C1C], np.uint16).tobytes(),
                             dtype=np.float16)[0])

    nc = bacc.Bacc(None, target_bir_lowering=False)
    names = {}

    with tile.TileContext(nc) as tc:
        with tc.tile_pool(name="dram", bufs=1, space="DRAM") as dram:
            xta = dram.tile([NIMG, 2, 64, PP], F16, kind="ExternalInput")
            xtb = dram.tile([NIMG, 2, 64, PP], F16, kind="ExternalInput")
            xtc = dram.tile([NIMG, 2, 64, PP], F16, kind="ExternalInput")
            xin = dram.tile([NIMG, 64, PIX], F32, kind="ExternalInput")
            w1a = dram.tile([128, 9 * 64], F16, kind="ExternalInput")
            w1p = dram.tile([128, 3 * 64], F16, kind="ExternalInput")
            w1s = dram.tile([128, 3 * 64], F16, kind="ExternalInput")
            w1q = dram.tile([128, 64], F16, kind="ExternalInput")
            w2d1 = dram.tile([128, 2, 9, 128], F8E4, kind="ExternalInput")
            w2d2 = dram.tile([128, 2, 9, 128], F8E4, kind="ExternalInput")
            cpar = dram.tile([128, 8], F32, kind="ExternalInput")
            ident = dram.tile([128, 128], F32, kind="ExternalInput")
            outp = dram.tile([NIMG, 64, PIX], F8E4, kind="ExternalOutput")
            names.update(xta=xta.name, xtb=xtb.name, xtc=xtc.name,
                         xin=xin.name,
                         w1a=w1a.name, w1p=w1p.name, w1s=w1s.name,
                         w1q=w1q.name,
                         w2d1=w2d1.name, w2d2=w2d2.name,
                         cpar=cpar.name, ident=ident.name,
                         outp=outp.name)

            with tc.tile_pool(name="dramw", bufs=1, space="DRAM") as dramw, \
                 tc.tile_pool(name="wsb", bufs=1) as wsb, \
                 tc.tile_pool(name="ys", bufs=8) as yspool, \
                 tc.tile_pool(name="plane", bufs=4) as plpool, \
                 tc.tile_pool(name="plb", bufs=2) as plbpool, \
                 tc.tile_pool(name="hfp", bufs=2) as hf, \
                 tc.tile_pool(name="tiny", bufs=8) as tiny, \
                 tc.tile_pool(name="ps", bufs=7, space="PSUM") as ps:

                w1as = wsb.tile([128, 9 * 64], F16, tag="w1a")
                nc.scalar.dma_start(w1as[:], w1a[:])
                w1ps = wsb.tile([128, 3 * 64], F16, tag="w1p")
                nc.scalar.dma_start(w1ps[:], w1p[:])
                w1ss = wsb.tile([128, 3 * 64], F16, tag="w1s")
                nc.scalar.dma_start(w1ss[:], w1s[:])
                w1qs = wsb.tile([128, 64], F16, tag="w1q")
                nc.scalar.dma_start(w1qs[:], w1q[:])
                w2d1s = wsb.tile([128, 2, 9, 128], F8E4, tag="w2d1")
                nc.scalar.dma_start(w2d1s[:], w2d1[:])
                w2d2s = wsb.tile([128, 2, 9, 128], F8E4, tag="w2d2")
                nc.scalar.dma_start(w2d2s[:], w2d2[:])
                cpars = wsb.tile([128, 8], F32, tag="cpar")
                nc.scalar.dma_start(cpars[:], cpar[:])
                idents = wsb.tile([128, 128], F32, tag="ident")
                nc.scalar.dma_start(idents[:], ident[:])
                sums1 = wsb.tile([128, 56], F32, tag="sums1")
                sums1q = wsb.tile([128, 56], F32, tag="sums1q")
                sums2 = wsb.tile([128, 56], F32, tag="sums2")
                sums2q = wsb.tile([128, 56], F32, tag="sums2q")
                scrv = wsb.tile([128, CHW], F32, tag="scrv")
                scra = wsb.tile([128, CHW], F32, tag="scra")
                sqwarm = tiny.tile([128, 1], F32, tag="t1")
                nc.vector.memset(sqwarm[:], 1.0)
                sqw2 = tiny.tile([128, 1], F32, tag="t1")
                nc.scalar.activation(sqw2[:], sqwarm[:], AF.Sqrt)

                def evac(pts, dst, sums_t, sumsq_t, col, on_act=False):
                    nc.scalar.activation(dst, pts[:], AF.Copy,
                                         accum_out=sums_t[:, col:col + 1])
                    if on_act:
                        nc.scalar.activation(
                            scra[:], dst, AF.Square,
                            accum_out=sumsq_t[:, col:col + 1])
                    else:
                        nc.vector.scalar_tensor_tensor(
                            scrv[:], dst, 1.0, dst, AO.bypass, AO.mult,
                            accum_out=sumsq_t[:, col:col + 1])

                def conv1_pair(plA, plB, pbA, pbB, pcA, pcB, dst_strip,
                               pcol):
                    plAr = plA.rearrange("p (r w) -> p r w", w=HP)
                    plBr = plB.rearrange("p (r w) -> p r w", w=HP)
                    pbAr = pbA.rearrange("p (r w) -> p r w", w=HP)
                    pbBr = pbB.rearrange("p (r w) -> p r w", w=HP)
                    pcAr = pcA.rearrange("p (r w) -> p r w", w=HP)
                    pcBr = pcB.rearrange("p (r w) -> p r w", w=HP)
                    for wave in (range(0, 4), range(4, 7)):
                        pts = {}
                        for cth in wave:
                            pts[cth] = ps.tile([128, CHW], F32, tag="ps",
                                               bufs=7, name=f"psum{cth}")
                        def hi_tap(a, start, stop):
                            di, dj = a // 3, a % 3
                            for cth in wave:
                                r0 = 8 * cth + di
                                for j, plr in enumerate((plAr, plBr)):
                                    out = pts[cth][64 * j:64 * (j + 1), :] \
                                        .rearrange("p (r w) -> p r w", r=8)
                                    nc.tensor.matmul(
                                        out, w1as[:, a * 64:(a + 1) * 64],
                                        plr[:, r0:r0 + 8, dj:dj + W],
                                        start=start, stop=stop,
                                        tile_position=(0, 64 * j),
                                        skip_group_check=True)

                        hi_tap(0, True, False)
                        for a in range(1, 5):
                            hi_tap(a, False, False)
                        for di in range(3):
                            for cth in wave:
                                r0 = 8 * cth + di
                                for j, pbr in enumerate((pbAr, pbBr)):
                                    out = pts[cth][64 * j:64 * (j + 1), :] \
                                        .rearrange("p (r w) -> p r w", r=8)
                                    nc.tensor.matmul(
                                        out, w1ps[:, di * 64:(di + 1) * 64],
                                        pbr[:, r0:r0 + 8, 0:W],
                                        start=False, stop=False,
                                        tile_position=(0, 64 * j),
                                        skip_group_check=True)
                        for cth in wave:
                            r0 = 8 * cth
                            for j, pcr in enumerate((pcAr, pcBr)):
                                out = pts[cth][64 * j:64 * (j + 1), :] \
                                    .rearrange("p (r w) -> p r w", r=8)
                                nc.tensor.matmul(
                                    out, w1qs[:, 0:64],
                                    pcr[:, r0:r0 + 8, 0:W],
                                    start=False, stop=False,
                                    tile_position=(0, 64 * j),
                                    skip_group_check=True)
                        for cth in wave:
                            r0 = 8 * cth + 2
                            for j, plr in enumerate((plAr, plBr)):
                                out = pts[cth][64 * j:64 * (j + 1), :] \
                                    .rearrange("p (r w) -> p r w", r=8)
                                nc.tensor.matmul(
                                    out, w1ss[0:64, 2 * 64:3 * 64],
                                    plr[0:64, r0:r0 + 8, 2:2 + W],
                                    start=False, stop=False,
                                    tile_position=(0, 64 * j),
                                    skip_group_check=True)
                        for a in range(5, 9):
                            hi_tap(a, False, a == 8)
                        for cth in wave:
                            evac(pts[cth],
                                 dst_strip[:, CHW * cth:CHW * (cth + 1)],
                                 sums1, sums1q, pcol * 7 + cth,
                                 on_act=(cth % 2 == 1))

                def conv2_pair(sp1, sp2, dst_strip, pcol):
                    p14 = sp1.rearrange("p (r w) k -> p k r w", w=HP)
                    p24 = sp2.rearrange("p (r w) k -> p k r w", w=HP)
                    for wave in (range(0, 4), range(4, 7)):
                        pts = {}
                        for cth in wave:
                            pts[cth] = ps.tile([128, CHW], F32, tag="ps",
                                               bufs=7, name=f"psum{cth}")
                        for pli, (pl4, wd) in enumerate(
                                ((p14, w2d1s), (p24, w2d2s))):
                            for a in range(9):
                                di, dj = a // 3, a % 3
                                for cth in wave:
                                    r0 = 8 * cth + di
                                    out = pts[cth][:].rearrange(
                                        "p (r w) -> p r w", r=8)
                                    nc.tensor.matmul(
                                        out, wd[:, :, a, :],
                                        pl4[:, :, r0:r0 + 8, dj:dj + W],
                                        start=(pli == 0 and a == 0),
                                        stop=(pli == 1 and a == 8),
                                        perf_mode=PM.DoubleRow,
                                        skip_group_check=True)
                        for cth in wave:
                            evac(pts[cth],
                                 dst_strip[:, CHW * cth:CHW * (cth + 1)],
                                 sums2, sums2q, pcol * 7 + cth,
                                 on_act=(cth % 2 == 1))

                y1s = []
                for p in range(NPAIR):
                    tt_, bp = p // 2, p % 2
                    iA = tt_ * 4 + bp * 2
                    tas, pbs, pcs = [], [], []
                    for j in range(2):
                        i = iA + j
                        ta = plpool.tile([128, PP], F16, tag="pl")
                        tar = ta.rearrange("p (r w) -> p r w", w=HP)
                        xtr = xta[i].rearrange("a c (r w) -> a c r w", w=HP)
                        nc.sync.dma_start(tar[:, 0:29, :],
                                          xtr[:, :, 0:29, :])
                        nc.gpsimd.dma_start(tar[:, 29:HP, :],
                                            xtr[:, :, 29:HP, :])
                        tas.append(ta)
                        pb = plbpool.tile([128, PP], F16, tag="plb",
                                          bufs=4)
                        nc.scalar.dma_start(pb[:], xtb[i])
                        pbs.append(pb)
                        pc = plbpool.tile([128, PP], F16, tag="plb",
                                          bufs=4)
                        nc.sync.dma_start(pc[:], xtc[i])
                        pcs.append(pc)
                    strip = yspool.tile([128, PIX], F32, tag=f"ys{p}",
                                        bufs=1)
                    y1s.append(strip)
                    conv1_pair(tas[0], tas[1], pbs[0], pbs[1], pcs[0],
                               pcs[1], strip, p)

                cc1i = dramw.tile([128, 2], F32)
                cc1o = dramw.tile([128, 2], F32, addr_space="Shared")
                acc1 = tiny.tile([128, 2], F32, tag="acc")
                nc.vector.tensor_reduce(acc1[:, 0:1], sums1[:], AX.X, AO.add)
                nc.vector.tensor_reduce(acc1[:, 1:2], sums1q[:], AX.X, AO.add)
                nc.sync.dma_start(cc1i[:], acc1[:])
                if NO_CC:
                    nc.sync.dma_start(cc1o[:], cc1i[:])
                else:
                    nc.gpsimd.collective_compute(
                        "AllReduce", AO.add, ins=[cc1i[:]], outs=[cc1o[:]],
                        replica_groups=[list(range(NCORES))])
                g1 = tiny.tile([128, 2], F32, tag="acc")
                nc.sync.dma_start(g1[:], cc1o[:])

                def stats_block(g, gamma, beta, rga, rgam, alpha):
                    totp = ps.tile([128, CHW], F32, tag="pstot", bufs=1,
                                   name="pstot")
                    nc.tensor.matmul(totp[:, 0:2], idents[:], g[:],
                                     start=True, stop=True,
                                     skip_group_check=True)
                    tot = totp[:, 0:2]
                    mm = tiny.tile([128, 2], F32, tag="acc")
                    nc.vector.tensor_scalar(mm[:], tot, 1.0 / NG, None,
                                            AO.mult)
                    mean = mm[:, 0:1]
                    m2 = tiny.tile([128, 1], F32, tag="t1")
                    nc.vector.tensor_tensor(m2[:], mean, mean, AO.mult)
                    var = tiny.tile([128, 1], F32, tag="t1")
                    nc.vector.tensor_tensor(var[:], mm[:, 1:2], m2[:],
                                            AO.subtract)
                    epst = tiny.tile([128, 1], F32, tag="t1")
                    nc.vector.memset(epst[:], EPS)
                    std = tiny.tile([128, 1], F32, tag="t1")
                    nc.scalar.activation(std[:], var[:], AF.Sqrt, bias=epst[:])
                    rstd = tiny.tile([128, 1], F32, tag="t1")
                    nc.vector.reciprocal(rstd[:], std[:])
                    rscv = tiny.tile([128, 1], F32, tag="t1")
                    nc.vector.tensor_tensor(rscv[:], std[:], rgam, AO.mult)
                    sc = tiny.tile([128, 1], F32, tag="t1")
                    nc.vector.tensor_tensor(sc[:], gamma, rstd[:], AO.mult)
                    nmsc = tiny.tile([128, 1], F32, tag="t1")
                    nc.vector.scalar_tensor_tensor(nmsc[:], mean, -1.0, sc[:],
                                                   AO.mult, AO.mult)
                    bi = tiny.tile([128, 1], F32, tag="t1")
                    nc.vector.tensor_tensor(bi[:], beta, nmsc[:], AO.add)
                    stdrg = tiny.tile([128, 1], F32, tag="t1")
                    nc.vector.tensor_tensor(stdrg[:], std[:], rga, AO.mult)
                    nbst = tiny.tile([128, 1], F32, tag="t1")
                    nc.vector.scalar_tensor_tensor(nbst[:], bi[:], -alpha,
                                                   stdrg[:], AO.mult, AO.mult)
                    th = tiny.tile([128, 1], F32, tag="t1")
                    nc.vector.tensor_tensor(th[:], stdrg[:], nbst[:], AO.add)
                    bstd = tiny.tile([128, 1], F32, tag="t1")
                    nc.vector.tensor_tensor(bstd[:], bi[:], std[:], AO.mult)
                    gamv = tiny.tile([128, 1], F32, tag="t1")
                    nc.vector.tensor_tensor(gamv[:], bstd[:], rgam, AO.mult)
                    gmw = tiny.tile([128, 1], F32, tag="t1")
                    nc.vector.tensor_scalar(gmw[:], gamv[:], 1.0 - alpha, None,
                                            AO.mult)
                    return th, gamv, rscv, gmw

                th1, gm1, _rsc1, gmw1 = stats_block(
                    g1, cpars[:, 0:1], cpars[:, 1:2], cpars[:, 4:5],
                    cpars[:, 6:7], alpha1)

                y2s = [None] * NPAIR
                for bp in range(2):
                    Pprev = [None] * NQ
                    for t in range(1, 5):
                        p = (t - 1) * 2 + bp
                        sp1 = plpool.tile([128, PP, 2], F8E5, tag="pl")
                        sp2 = plpool.tile([128, PP, 2], F8E5, tag="pl")
                        w1v = sp1.rearrange("p (r w) k -> p r w k", w=HP)
                        w2v = sp2.rearrange("p (r w) k -> p r w k", w=HP)
                        for sp4 in (w1v, w2v):
                            nc.gpsimd.memset(sp4[:, 0:1, :, :], 0.0)
                            nc.gpsimd.memset(sp4[:, HP - 1:HP, :, :], 0.0)
                            nc.gpsimd.memset(sp4[:, 1:HP - 1, 0:1, :], 0.0)
                            nc.gpsimd.memset(sp4[:, 1:HP - 1,
                                                 HP - 1:HP, :], 0.0)
                        for hq in range(NQ):
                            off = QL * hq
                            ysl = y1s[p][:, off:off + QL]
                            if t == 1:
                                qa = ysl
                            else:
                                q = hf.tile([128, QL], F32, tag="q2", bufs=4)
                                if hq % 2 == 0:
                                    nc.vector.affine_then_add(
                                        q[:], Pprev[hq][:], ysl, 1.0, 0.0)
                                else:
                                    nc.gpsimd.tensor_tensor(
                                        q[:], ysl, Pprev[hq][:], AO.add)
                                qa = q[:]
                            qa3 = qa.rearrange("p (r w) -> p r w", w=W)
                            r0, r1_ = 1 + 14 * hq, 15 + 14 * hq
                            d1 = w1v[:, r0:r1_, 1:1 + W, :].bitcast(F16)
                            d2 = w2v[:, r0:r1_, 1:1 + W, :].bitcast(F16)
                            seng = nc.vector if hq % 2 == 0 else nc.gpsimd
                            nc.vector.tensor_scalar(d1, qa3, th1[:],
                                                    V1, AO.is_ge, AO.mult)
                            seng.tensor_scalar(d2, qa3, th1[:],
                                               V2, AO.is_ge, AO.mult)
                            if t < 4:
                                wv = hf.tile([128, QL], F32, tag="wv",
                                             bufs=2)
                                nc.scalar.activation(wv[:], qa, AF.Identity,
                                                     bias=gmw1[:],
                                                     scale=1.0 - alpha1)
                                Pn = hf.tile([128, QL], F32, tag="pp",
                                             bufs=4)
                                nc.vector.scalar_tensor_tensor(
                                    Pn[:], qa, th1[:], wv[:],
                                    AO.is_lt, AO.mult)
                                Pprev[hq] = Pn
                        strip2 = yspool.tile([128, PIX], F32, tag=f"ys{p}",
                                             bufs=1)
                        y2s[p] = strip2
                        conv2_pair(sp1, sp2, strip2, p)

                cc2i = dramw.tile([128, 2], F32)
                cc2o = dramw.tile([128, 2], F32, addr_space="Shared")
                acc2 = tiny.tile([128, 2], F32, tag="acc")
                nc.vector.tensor_reduce(acc2[:, 0:1], sums2[:], AX.X, AO.add)
                nc.vector.tensor_reduce(acc2[:, 1:2], sums2q[:], AX.X, AO.add)
                nc.sync.dma_start(cc2i[:], acc2[:])
                if NO_CC:
                    nc.sync.dma_start(cc2o[:], cc2i[:])
                else:
                    nc.gpsimd.collective_compute(
                        "AllReduce", AO.add, ins=[cc2i[:]], outs=[cc2o[:]],
                        replica_groups=[list(range(NCORES))])
                g2 = tiny.tile([128, 2], F32, tag="acc")
                nc.sync.dma_start(g2[:], cc2o[:])
                th2, gm2, rsc2, gmw2 = stats_block(
                    g2, cpars[:, 2:3], cpars[:, 3:4], cpars[:, 5:6],
                    cpars[:, 7:8], alpha2)
                nth2 = tiny.tile([128, 1], F32, tag="t1")
                nc.vector.tensor_scalar(nth2[:], th2[:], -1.0, None, AO.mult)
                gw2 = tiny.tile([128, 1], F32, tag="t1")
                nc.vector.scalar_tensor_tensor(gw2[:], th2[:], 1.0 - alpha2,
                                               gmw2[:], AO.mult, AO.add)

                PYprev = {0: [None] * NQ, 1: [None] * NQ}
                iters = [(t, bp, hq) for t in range(1, 5)
                         for bp in range(2) for hq in range(NQ)]
                KPF = 2
                xstiles = {}

                def issue_load(idx):
                    t, bp, hq = iters[idx]
                    iA = (t - 1) * 4 + bp * 2
                    off = QL * hq
                    xs = hf.tile([128, QL], F32, tag="xs", bufs=4)
                    nc.sync.dma_start(xs[:],
                                      xin[iA:iA + 2, :, off:off + QL])
                    xstiles[idx] = xs

                wvtiles = {}
                ottiles = {}

                def flush_tail(idx):
                    t, bp, hq = iters[idx]
                    iA = (t - 1) * 4 + bp * 2
                    off = QL * hq
                    xs = xstiles.pop(idx)
                    if t < 4:
                        ptag = ("pp", 4) if bp == 0 else ("q2", 4)
                        Pn = hf.tile([128, QL], F32, tag=ptag[0],
                                     bufs=ptag[1])
                        nc.vector.scalar_tensor_tensor(
                            Pn[:], xs[:], 0.0, wvtiles.pop(idx)[:],
                            AO.is_lt, AO.mult)
                        p_next = t * 2 + bp
                        nc.gpsimd.tensor_tensor(
                            Pn[:], Pn[:], y2s[p_next][:, off:off + QL],
                            AO.add)
                        PYprev[bp][hq] = Pn
                    nc.gpsimd.dma_start(outp[iA:iA + 2, :, off:off + QL],
                                        ottiles.pop(idx)[:])

                for idx in range(KPF):
                    issue_load(idx)
                for idx, (t, bp, hq) in enumerate(iters):
                    if idx + KPF < len(iters):
                        issue_load(idx + KPF)
                    p = (t - 1) * 2 + bp
                    off = QL * hq
                    xs = xstiles[idx]
                    py = y2s[p][:, off:off + QL] if t == 1 \
                        else PYprev[bp][hq][:]
                    nc.vector.affine_then_add(xs[:], xs[:], py,
                                              rsc2[:], nth2[:])
                    if t < 4:
                        wv2 = hf.tile([128, QL], F32, tag="wv", bufs=2)
                        nc.scalar.activation(wv2[:], xs[:], AF.Identity,
                                             bias=gw2[:],
                                             scale=1.0 - alpha2)
                        wvtiles[idx] = wv2
                    ot = hf.tile([128, QL], F8E4, tag="ot", bufs=2)
                    nc.scalar.activation(ot[:], xs[:], AF.Sign)
                    ottiles[idx] = ot
                    if idx > 0:
                        flush_tail(idx - 1)
                flush_tail(len(iters) - 1)

    nc.compile()
    return nc, names


def _sigmoid(x):
    return 1.0 / (1.0 + np.exp(-float(x)))


def prepare(x, conv1_w, bn1_gamma, bn1_beta, lif1_w, conv2_w, bn2_gamma,
            bn2_beta, lif2_w):
    x = np.ascontiguousarray(np.asarray(x, np.float32))
    conv1_w = np.asarray(conv1_w, np.float32)
    conv2_w = np.asarray(conv2_w, np.float32)

    a1 = _sigmoid(np.asarray(lif1_w).reshape(-1)[0])
    a2 = _sigmoid(np.asarray(lif2_w).reshape(-1)[0])

    key = (round(a1, 12), round(a2, 12))
    if key not in _prog_cache:
        _prog_cache[key] = _build(a1, a2)
    nc, names = _prog_cache[key]

    xh = x.astype(np.float16)
    xl = (x - xh.astype(np.float32)).astype(np.float16)
    xpad = np.zeros((T, B, C, 2, HP, HP), np.float16)
    xpad[:, :, :, 0, 1:57, 1:57] = xh
    xpad[:, :, :, 1, 1:57, 1:57] = xl
    xpad = np.ascontiguousarray(xpad.transpose(0, 1, 3, 2, 4, 5))

    xhf = xpad[:, :, 0].reshape(T, B, C, PP)
    xbs = np.zeros((T, B, 2, C, PP), np.float16)
    xbs[:, :, 0] = xhf
    xbs[:, :, 1, :, :PP - 1] = xhf[:, :, :, 1:]
    xcs = np.zeros((T, B, 2, C, PP), np.float16)
    xcs[:, :, 0, :, :PP - 2] = xhf[:, :, :, 2:]
    xcs[:, :, 1, :, :PP - HP - 2] = xhf[:, :, :, HP + 2:]

    w1h = conv1_w.astype(np.float16)
    w1l = (conv1_w - w1h.astype(np.float32)).astype(np.float16)

    def tapstack(wtop, wbot):
        out = np.zeros((128, 9 * 64), np.float16)
        for a in range(9):
            di, dj = a // 3, a % 3
            out[0:64, a * 64:(a + 1) * 64] = wtop[:, :, di, dj].T
            out[64:128, a * 64:(a + 1) * 64] = wbot[:, :, di, dj].T
        return out

    w1a_np = tapstack(w1h, w1h)
    w1q_np = np.zeros((128, 64), np.float16)
    w1q_np[0:64, :] = w1l[:, :, 0, 2].T
    w1q_np[64:128, :] = w1l[:, :, 1, 2].T
    w1p_np = np.zeros((128, 3 * 64), np.float16)
    w1s_np = np.zeros((128, 3 * 64), np.float16)
    for di in range(3):
        w1p_np[0:64, di * 64:(di + 1) * 64] = w1l[:, :, di, 0].T
        w1p_np[64:128, di * 64:(di + 1) * 64] = w1l[:, :, di, 1].T
        w1s_np[0:64, di * 64:(di + 1) * 64] = w1l[:, :, di, 2].T

    w0 = conv2_w.astype(E4)
    r1 = conv2_w - w0.astype(np.float32)
    w1t = (r1 * 16).astype(E4)
    r2 = r1 - w1t.astype(np.float32) / 16
    w2t = (r2 * 256).astype(E4)
    r3 = r2 - w2t.astype(np.float32) / 256
    w3t = (r3 * 4096).astype(E4)
    w2d1_np = np.zeros((128, 2, 9, 128), E4)
    w2d2_np = np.zeros((128, 2, 9, 128), E4)
    for a in range(9):
        di, dj = a // 3, a % 3
        for blk in range(2):
            sl = slice(64 * blk, 64 * blk + 64)
            w2d1_np[sl, 0, a, sl] = w0[:, :, di, dj].T
            w2d1_np[sl, 1, a, sl] = w1t[:, :, di, dj].T
            w2d2_np[sl, 0, a, sl] = w2t[:, :, di, dj].T
            w2d2_np[sl, 1, a, sl] = w3t[:, :, di, dj].T

    def dup(v):
        v = np.asarray(v, np.float32).reshape(64)
        return np.concatenate([v, v])

    cpar_np = np.zeros((128, 8), np.float32)
    cpar_np[:, 0] = dup(bn1_gamma)
    cpar_np[:, 1] = dup(bn1_beta)
    cpar_np[:, 2] = dup(bn2_gamma)
    cpar_np[:, 3] = dup(bn2_beta)
    cpar_np[:, 4] = 1.0 / (a1 * dup(bn1_gamma))
    cpar_np[:, 5] = 1.0 / (a2 * dup(bn2_gamma))
    cpar_np[:, 6] = 1.0 / dup(bn1_gamma)
    cpar_np[:, 7] = 1.0 / dup(bn2_gamma)

    kk, mm_ = np.meshgrid(np.arange(128), np.arange(128), indexing='ij')
    ident_np = (kk % 64 == mm_ % 64).astype(np.float32)

    in_maps = []
    for k in range(NCORES):
        xta_np = np.ascontiguousarray(
            xpad[:, 4 * k:4 * k + 4].reshape(NIMG, 2, 64, PP))
        xtb_np = np.ascontiguousarray(
            xbs[:, 4 * k:4 * k + 4].reshape(NIMG, 2, 64, PP))
        xtc_np = np.ascontiguousarray(
            xcs[:, 4 * k:4 * k + 4].reshape(NIMG, 2, 64, PP))
        xin_np = np.ascontiguousarray(
            x[:, 4 * k:4 * k + 4].reshape(NIMG, 64, PIX))
        in_maps.append({
            names['xta']: xta_np,
            names['xtb']: xtb_np,
            names['xtc']: xtc_np,
            names['w1q']: w1q_np,
            names['xin']: xin_np,
            names['w1a']: w1a_np,
            names['w1p']: w1p_np,
            names['w1s']: w1s_np,
            names['w2d1']: w2d1_np,
            names['w2d2']: w2d2_np,
            names['cpar']: cpar_np,
            names['ident']: ident_np,
        })

    return nc, names, in_maps


def kernel(**inputs):
    from concourse.bass_utils import run_bass_kernel_spmd
    nc, names, in_maps = prepare(**inputs)
    res = run_bass_kernel_spmd(nc, in_maps, core_ids=list(range(NCORES)))
    global LAST_RES, LAST_NAMES
    LAST_RES, LAST_NAMES = res, names
    out = np.empty((T, B, C, H, W), np.float32)
    for k in range(NCORES):
        o = res.results[k][names['outp']].astype(np.float32)
        o = (o >= -0.5).astype(np.float32)
        out[:, 4 * k:4 * k + 4] = o.reshape(T, BL, C, H, W)
    return out


if __name__ == "__main__":
    rng = np.random.default_rng(0)
    xs = rng.standard_normal((T, B, C, H, W)).astype(np.float32)
    w1 = (rng.standard_normal((64, 64, 3, 3)) * 0.05).astype(np.float32)
    w2 = (rng.standard_normal((64, 64, 3, 3)) * 0.05).astype(np.float32)
    o = kernel(x=xs, conv1_w=w1, bn1_gamma=np.ones(64, np.float32),
               bn1_beta=np.zeros(64, np.float32),
               lif1_w=np.zeros(1, np.float32), conv2_w=w2,
               bn2_gamma=np.ones(64, np.float32),
               bn2_beta=np.zeros(64, np.float32),
               lif2_w=np.zeros(1, np.float32))
    print("ran:", o.shape, float(o.mean()))


# revision 20
# speedup vs baseline: 1.3357x; 1.1376x over previous
import sys
sys.path.insert(0, '/opt/trn_rl_repo')

import numpy as np
import ml_dtypes

T, B, C, H, W = 4, 32, 64, 56, 56
NCORES = 8
BL = B // NCORES
NIMG = T * BL
HP = W + 2
PP = HP * HP
PIX = H * W
NCH = 7
CHW = 8 * W
NPAIR = 8
EPS = 1e-5
NG = float((T * B) * PIX)
QL = 14 * W
NQ = 4
E4 = ml_dtypes.float8_e4m3
E5 = ml_dtypes.float8_e5m2

_prog_cache = {}
NO_CC = False
TRACE = False
LAST_RES = None
LAST_NAMES = None


def _build(alpha1, alpha2):
    import concourse.mybir as mybir
    import concourse.tile as tile
    from concourse import bacc

    F32 = mybir.dt.float32
    F16 = mybir.dt.float16
    F8E4 = mybir.dt.float8e4
    F8E5 = mybir.dt.float8e5
    AO = mybir.AluOpType
    AF = mybir.ActivationFunctionType
    AX = mybir.AxisListType
    PM = mybir.MatmulPerfMode

    V1 = float(np.frombuffer(np.array([0x2C3C], np.uint16).tobytes(),
                             dtype=np.float16)[0])
    V2 = float(np.frombuffer(np.array([0x0C1C], np.uint16).tobytes(),
                             dtype=np.float16)[0])

    nc = bacc.Bacc(None, target_bir_lowering=False)
    names = {}

    with tile.TileContext(nc) as tc:
        with tc.tile_pool(name="dram", bufs=1, space="DRAM") as dram:
            xta = dram.tile([NIMG, 2, 64, PP], F16, kind="ExternalInput")
            xtb = dram.tile([NIMG, 2, 64, PP], F16, kind="ExternalInput")
            xtc = dram.tile([NIMG, 2, 64, PP], F16, kind="ExternalInput")
            xin = dram.tile([NIMG, 64, PIX], F32, kind="ExternalInput")
            w1a = dram.tile([128, 9 * 64], F16, kind="ExternalInput")
            w1p = dram.tile([128, 3 * 64], F16, kind="ExternalInput")
            w1s = dram.tile([128, 3 * 64], F16, kind="ExternalInput")
            w1q = dram.tile([128, 64], F16, kind="ExternalInput")
            w2d1 = dram.tile([128, 2, 9, 128], F8E4, kind="ExternalInput")
            w2d2 = dram.tile([128, 2, 9, 128], F8E4, kind="ExternalInput")
            cpar = dram.tile([128, 8], F32, kind="ExternalInput")
            ident = dram.tile([128, 128], F32, kind="ExternalInput")
            outp = dram.tile([NIMG, 64, PIX], F8E4, kind="ExternalOutput")
            names.update(xta=xta.name, xtb=xtb.name, xtc=xtc.name,
                         xin=xin.name,
                         w1a=w1a.name, w1p=w1p.name, w1s=w1s.name,
                         w1q=w1q.name,
                         w2d1=w2d1.name, w2d2=w2d2.name,
                         cpar=cpar.name, ident=ident.name,
                         outp=outp.name)

            with tc.tile_pool(name="dramw", bufs=1, space="DRAM") as dramw, \
                 tc.tile_pool(name="wsb", bufs=1) as wsb, \
                 tc.tile_pool(name="ys", bufs=8) as yspool, \
                 tc.tile_pool(name="plane", bufs=4) as plpool, \
                 tc.tile_pool(name="plb", bufs=2) as plbpool, \
                 tc.tile_pool(name="hfp", bufs=2) as hf, \
                 tc.tile_pool(name="tiny", bufs=8) as tiny, \
                 tc.tile_pool(name="ps", bufs=7, space="PSUM") as ps:

                w1as = wsb.tile([128, 9 * 64], F16, tag="w1a")
                nc.scalar.dma_start(w1as[:], w1a[:])
                w1ps = wsb.tile([128, 3 * 64], F16, tag="w1p")
                nc.scalar.dma_start(w1ps[:], w1p[:])
                w1ss = wsb.tile([128, 3 * 64], F16, tag="w1s")
                nc.scalar.dma_start(w1ss[:], w1s[:])
                w1qs = wsb.tile([128, 64], F16, tag="w1q")
                nc.scalar.dma_start(w1qs[:], w1q[:])
                w2d1s = wsb.tile([128, 2, 9, 128], F8E4, tag="w2d1")
                nc.scalar.dma_start(w2d1s[:], w2d1[:])
                w2d2s = wsb.tile([128, 2, 9, 128], F8E4, tag="w2d2")
                nc.scalar.dma_start(w2d2s[:], w2d2[:])
                cpars = wsb.tile([128, 8], F32, tag="cpar")
                nc.scalar.dma_start(cpars[:], cpar[:])
                idents = wsb.tile([128, 128], F32, tag="ident")
                nc.scalar.dma_start(idents[:], ident[:])
                sums1 = wsb.tile([128, 56], F32, tag="sums1")
                sums1q = wsb.tile([128, 56], F32, tag="sums1q")
                sums2 = wsb.tile([128, 56], F32, tag="sums2")
                sums2q = wsb.tile([128, 56], F32, tag="sums2q")
                scrv = wsb.tile([128, CHW], F32, tag="scrv")
                scra = wsb.tile([128, CHW], F32, tag="scra")
                sqwarm = tiny.tile([128, 1], F32, tag="t1")
                nc.vector.memset(sqwarm[:], 1.0)
                sqw2 = tiny.tile([128, 1], F32, tag="t1")
                nc.scalar.activation(sqw2[:], sqwarm[:], AF.Sqrt)

                def evac(pts, dst, sums_t, sumsq_t, col, on_act=False):
                    nc.scalar.activation(dst, pts[:], AF.Copy,
                                         accum_out=sums_t[:, col:col + 1])
                    if on_act:
                        nc.scalar.activation(
                            scra[:], dst, AF.Square,
                            accum_out=sumsq_t[:, col:col + 1])
                    else:
                        nc.vector.scalar_tensor_tensor(
                            scrv[:], dst, 1.0, dst, AO.bypass, AO.mult,
                            accum_out=sumsq_t[:, col:col + 1])

                def conv1_pair(plA, plB, pbA, pbB, pcA, pcB, dst_strip,
                               pcol):
                    plAr = plA.rearrange("p (r w) -> p r w", w=HP)
                    plBr = plB.rearrange("p (r w) -> p r w", w=HP)
                    pbAr = pbA.rearrange("p (r w) -> p r w", w=HP)
                    pbBr = pbB.rearrange("p (r w) -> p r w", w=HP)
                    pcAr = pcA.rearrange("p (r w) -> p r w", w=HP)
                    pcBr = pcB.rearrange("p (r w) -> p r w", w=HP)
                    for wave in (range(0, 4), range(4, 7)):
                        pts = {}
                        for cth in wave:
                            pts[cth] = ps.tile([128, CHW], F32, tag="ps",
                                               bufs=7, name=f"psum{cth}")
                        def hi_tap(a, start, stop):
                            di, dj = a // 3, a % 3
                            for cth in wave:
                                r0 = 8 * cth + di
                                for j, plr in enumerate((plAr, plBr)):
                                    out = pts[cth][64 * j:64 * (j + 1), :] \
                                        .rearrange("p (r w) -> p r w", r=8)
                                    nc.tensor.matmul(
                                        out, w1as[:, a * 64:(a + 1) * 64],
                                        plr[:, r0:r0 + 8, dj:dj + W],
                                        start=start, stop=stop,
                                        tile_position=(0, 64 * j),
                                        skip_group_check=True)

                        hi_tap(0, True, False)
                        for a in range(1, 5):
                            hi_tap(a, False, False)
                        for di in range(3):
                            for cth in wave:
                                r0 = 8 * cth + di
                                for j, pbr in enumerate((pbAr, pbBr)):
                                    out = pts[cth][64 * j:64 * (j + 1), :] \
                                        .rearrange("p (r w) -> p r w", r=8)
                                    nc.tensor.matmul(
                                        out, w1ps[:, di * 64:(di + 1) * 64],
                                        pbr[:, r0:r0 + 8, 0:W],
                                        start=False, stop=False,
                                        tile_position=(0, 64 * j),
                                        skip_group_check=True)
                        for cth in wave:
                            r0 = 8 * cth
                            for j, pcr in enumerate((pcAr, pcBr)):
                                out = pts[cth][64 * j:64 * (j + 1), :] \
                                    .rearrange("p (r w) -> p r w", r=8)
                                nc.tensor.matmul(
                                    out, w1qs[:, 0:64],
                                    pcr[:, r0:r0 + 8, 0:W],
                                    start=False, stop=False,
                                    tile_position=(0, 64 * j),
                                    skip_group_check=True)
                        for cth in wave:
                            r0 = 8 * cth + 2
                            for j, plr in enumerate((plAr, plBr)):
                                out = pts[cth][64 * j:64 * (j + 1), :] \
                                    .rearrange("p (r w) -> p r w", r=8)
                                nc.tensor.matmul(
                                    out, w1ss[0:64, 2 * 64:3 * 64],
                                    plr[0:64, r0:r0 + 8, 2:2 + W],
                                    start=False, stop=False,
                                    tile_position=(0, 64 * j),
                                    skip_group_check=True)
                        for a in range(5, 9):
                            hi_tap(a, False, a == 8)
                        for cth in wave:
                            evac(pts[cth],
                                 dst_strip[:, CHW * cth:CHW * (cth + 1)],
                                 sums1, sums1q, pcol * 7 + cth,
                                 on_act=(cth % 2 == 1))

                def conv2_pair(sp1, sp2, dst_strip, pcol):
                    p14 = sp1.rearrange("p (r w) k -> p k r w", w=HP)
                    p24 = sp2.rearrange("p (r w) k -> p k r w", w=HP)
                    for wave in (range(0, 4), range(4, 7)):
                        pts = {}
                        for cth in wave:
                            pts[cth] = ps.tile([128, CHW], F32, tag="ps",
                                               bufs=7, name=f"psum{cth}")
                        for pli, (pl4, wd) in enumerate(
                                ((p14, w2d1s), (p24, w2d2s))):
                            for a in range(9):
                                di, dj = a // 3, a % 3
                                for cth in wave:
                                    r0 = 8 * cth + di
                                    out = pts[cth][:].rearrange(
                                        "p (r w) -> p r w", r=8)
                                    nc.tensor.matmul(
                                        out, wd[:, :, a, :],
                                        pl4[:, :, r0:r0 + 8, dj:dj + W],
                                        start=(pli == 0 and a == 0),
                                        stop=(pli == 1 and a == 8),
                                        perf_mode=PM.DoubleRow,
                                        skip_group_check=True)
                        for cth in wave:
                            evac(pts[cth],
                                 dst_strip[:, CHW * cth:CHW * (cth + 1)],
                                 sums2, sums2q, pcol * 7 + cth,
                                 on_act=(cth % 2 == 1))

                y1s = []
                for p in range(NPAIR):
                    tt_, bp = p // 2, p % 2
                    iA = tt_ * 4 + bp * 2
                    tas, pbs, pcs = [], [], []
                    for j in range(2):
                        i = iA + j
                        ta = plpool.tile([128, PP], F16, tag="pl")
                        tar = ta.rearrange("p (r w) -> p r w", w=HP)
                        xtr = xta[i].rearrange("a c (r w) -> a c r w", w=HP)
                        nc.sync.dma_start(tar[:, 0:29, :],
                                          xtr[:, :, 0:29, :])
                        nc.scalar.dma_start(tar[:, 29:HP, :],
                                            xtr[:, :, 29:HP, :])
                        tas.append(ta)
                        pb = plbpool.tile([128, PP], F16, tag="plb",
                                          bufs=4)
                        nc.scalar.dma_start(pb[:], xtb[i])
                        pbs.append(pb)
                        pc = plbpool.tile([128, PP], F16, tag="plb",
                                          bufs=4)
                        nc.sync.dma_start(pc[:], xtc[i])
                        pcs.append(pc)
                    strip = yspool.tile([128, PIX], F32, tag=f"ys{p}",
                                        bufs=1)
                    y1s.append(strip)
                    conv1_pair(tas[0], tas[1], pbs[0], pbs[1], pcs[0],
                               pcs[1], strip, p)

                cc1i = dramw.tile([128, 2], F32)
                cc1o = dramw.tile([128, 2], F32, addr_space="Shared")
                acc1 = tiny.tile([128, 2], F32, tag="acc")
                nc.vector.tensor_reduce(acc1[:, 0:1], sums1[:], AX.X, AO.add)
                nc.vector.tensor_reduce(acc1[:, 1:2], sums1q[:], AX.X, AO.add)
                nc.sync.dma_start(cc1i[:], acc1[:])
                if NO_CC:
                    nc.sync.dma_start(cc1o[:], cc1i[:])
                else:
                    nc.gpsimd.collective_compute(
                        "AllReduce", AO.add, ins=[cc1i[:]], outs=[cc1o[:]],
                        replica_groups=[list(range(NCORES))])
                g1 = tiny.tile([128, 2], F32, tag="acc")
                nc.sync.dma_start(g1[:], cc1o[:])

                def stats_block(g, gamma, beta, rga, rgam, alpha):
                    totp = ps.tile([128, CHW], F32, tag="pstot", bufs=1,
                                   name="pstot")
                    nc.tensor.matmul(totp[:, 0:2], idents[:], g[:],
                                     start=True, stop=True,
                                     skip_group_check=True)
                    tot = totp[:, 0:2]
                    mm = tiny.tile([128, 2], F32, tag="acc")
                    nc.vector.tensor_scalar(mm[:], tot, 1.0 / NG, None,
                                            AO.mult)
                    mean = mm[:, 0:1]
                    m2 = tiny.tile([128, 1], F32, tag="t1")
                    nc.vector.tensor_tensor(m2[:], mean, mean, AO.mult)
                    var = tiny.tile([128, 1], F32, tag="t1")
                    nc.vector.tensor_tensor(var[:], mm[:, 1:2], m2[:],
                                            AO.subtract)
                    epst = tiny.tile([128, 1], F32, tag="t1")
                    nc.vector.memset(epst[:], EPS)
                    std = tiny.tile([128, 1], F32, tag="t1")
                    nc.scalar.activation(std[:], var[:], AF.Sqrt, bias=epst[:])
                    rstd = tiny.tile([128, 1], F32, tag="t1")
                    nc.vector.reciprocal(rstd[:], std[:])
                    rscv = tiny.tile([128, 1], F32, tag="t1")
                    nc.vector.tensor_tensor(rscv[:], std[:], rgam, AO.mult)
                    sc = tiny.tile([128, 1], F32, tag="t1")
                    nc.vector.tensor_tensor(sc[:], gamma, rstd[:], AO.mult)
                    nmsc = tiny.tile([128, 1], F32, tag="t1")
                    nc.vector.scalar_tensor_tensor(nmsc[:], mean, -1.0, sc[:],
                                                   AO.mult, AO.mult)
                    bi = tiny.tile([128, 1], F32, tag="t1")
                    nc.vector.tensor_tensor(bi[:], beta, nmsc[:], AO.add)
                    stdrg = tiny.tile([128, 1], F32, tag="t1")
                    nc.vector.tensor_tensor(stdrg[:], std[:], rga, AO.mult)
                    nbst = tiny.tile([128, 1], F32, tag="t1")
                    nc.vector.scalar_tensor_tensor(nbst[:], bi[:], -alpha,
                                                   stdrg[:], AO.mult, AO.mult)
                    th = tiny.tile([128, 1], F32, tag="t1")
                    nc.vector.tensor_tensor(th[:], stdrg[:], nbst[:], AO.add)
                    bstd = tiny.tile([128, 1], F32, tag="t1")
                    nc.vector.tensor_tensor(bstd[:], bi[:], std[:], AO.mult)
                    gamv = tiny.tile([128, 1], F32, tag="t1")
                    nc.vector.tensor_tensor(gamv[:], bstd[:], rgam, AO.mult)
                    gmw = tiny.tile([128, 1], F32, tag="t1")
                    nc.vector.tensor_scalar(gmw[:], gamv[:], 1.0 - alpha, None,
                                            AO.mult)
                    return th, gamv, rscv, gmw

                th1, gm1, _rsc1, gmw1 = stats_block(
                    g1, cpars[:, 0:1], cpars[:, 1:2], cpars[:, 4:5],
                    cpars[:, 6:7], alpha1)

                y2s = [None] * NPAIR
                for bp in range(2):
                    Pprev = [None] * NQ
                    for t in range(1, 5):
                        p = (t - 1) * 2 + bp
                        sp1 = plpool.tile([128, PP, 2], F8E5, tag="pl")
                        sp2 = plpool.tile([128, PP, 2], F8E5, tag="pl")
                        w1v = sp1.rearrange("p (r w) k -> p r w k", w=HP)
                        w2v = sp2.rearrange("p (r w) k -> p r w k", w=HP)
                        for sp4 in (w1v, w2v):
                            nc.gpsimd.memset(sp4[:, 0:1, :, :], 0.0)
                            nc.gpsimd.memset(sp4[:, HP - 1:HP, :, :], 0.0)
                            nc.gpsimd.memset(sp4[:, 1:HP - 1, 0:1, :], 0.0)
                            nc.gpsimd.memset(sp4[:, 1:HP - 1,
                                                 HP - 1:HP, :], 0.0)
                        for hq in range(NQ):
                            off = QL * hq
                            ysl = y1s[p][:, off:off + QL]
                            if t == 1:
                                qa = ysl
                            else:
                                q = hf.tile([128, QL], F32, tag="q2", bufs=4)
                                if hq % 2 == 0:
                                    nc.vector.affine_then_add(
                                        q[:], Pprev[hq][:], ysl, 1.0, 0.0)
                                else:
                                    nc.gpsimd.tensor_tensor(
                                        q[:], ysl, Pprev[hq][:], AO.add)
                                qa = q[:]
                            qa3 = qa.rearrange("p (r w) -> p r w", w=W)
                            r0, r1_ = 1 + 14 * hq, 15 + 14 * hq
                            d1 = w1v[:, r0:r1_, 1:1 + W, :].bitcast(F16)
                            d2 = w2v[:, r0:r1_, 1:1 + W, :].bitcast(F16)
                            seng = nc.vector if hq % 2 == 0 else nc.gpsimd
                            nc.vector.tensor_scalar(d1, qa3, th1[:],
                                                    V1, AO.is_ge, AO.mult)
                            seng.tensor_scalar(d2, qa3, th1[:],
                                               V2, AO.is_ge, AO.mult)
                            if t < 4:
                                wv = hf.tile([128, QL], F32, tag="wv",
                                             bufs=2)
                                nc.scalar.activation(wv[:], qa, AF.Identity,
                                                     bias=gmw1[:],
                                                     scale=1.0 - alpha1)
                                Pn = hf.tile([128, QL], F32, tag="pp",
                                             bufs=4)
                                nc.vector.scalar_tensor_tensor(
                                    Pn[:], qa, th1[:], wv[:],
                                    AO.is_lt, AO.mult)
                                Pprev[hq] = Pn
                        strip2 = yspool.tile([128, PIX], F32, tag=f"ys{p}",
                                             bufs=1)
                        y2s[p] = strip2
                        conv2_pair(sp1, sp2, strip2, p)

                cc2i = dramw.tile([128, 2], F32)
                cc2o = dramw.tile([128, 2], F32, addr_space="Shared")
                acc2 = tiny.tile([128, 2], F32, tag="acc")
                nc.vector.tensor_reduce(acc2[:, 0:1], sums2[:], AX.X, AO.add)
                nc.vector.tensor_reduce(acc2[:, 1:2], sums2q[:], AX.X, AO.add)
                nc.sync.dma_start(cc2i[:], acc2[:])
                if NO_CC:
                    nc.sync.dma_start(cc2o[:], cc2i[:])
                else:
                    nc.gpsimd.collective_compute(
                        "AllReduce", AO.add, ins=[cc2i[:]], outs=[cc2o[:]],
                        replica_groups=[list(range(NCORES))])
                g2 = tiny.tile([128, 2], F32, tag="acc")
                nc.sync.dma_start(g2[:], cc2o[:])
                th2, gm2, rsc2, gmw2 = stats_block(
                    g2, cpars[:, 2:3], cpars[:, 3:4], cpars[:, 5:6],
                    cpars[:, 7:8], alpha2)
                nth2 = tiny.tile([128, 1], F32, tag="t1")
                nc.vector.tensor_scalar(nth2[:], th2[:], -1.0, None, AO.mult)
                gw2 = tiny.tile([128, 1], F32, tag="t1")
                nc.vector.scalar_tensor_tensor(gw2[:], th2[:], 1.0 - alpha2,
                                               gmw2[:], AO.mult, AO.add)

                PYprev = {0: [None] * NQ, 1: [None] * NQ}
                iters = [(t, bp, hq) for t in range(1, 5)
                         for bp in range(2) for hq in range(NQ)]
                KPF = 2
                xstiles = {}

                def issue_load(idx):
                    t, bp, hq = iters[idx]
                    iA = (t - 1) * 4 + bp * 2
                    off = QL * hq
                    xs = hf.tile([128, QL], F32, tag="xs", bufs=4)
                    nc.sync.dma_start(xs[:],
                                      xin[iA:iA + 2, :, off:off + QL])
                    xstiles[idx] = xs

                wvtiles = {}
                ottiles = {}

                def flush_tail(idx):
                    t, bp, hq = iters[idx]
                    iA = (t - 1) * 4 + bp * 2
                    off = QL * hq
                    xs = xstiles.pop(idx)
                    if t < 4:
                        ptag = ("pp", 4) if bp == 0 else ("q2", 4)
                        Pn = hf.tile([128, QL], F32, tag=ptag[0],
                                     bufs=ptag[1])
                        nc.vector.scalar_tensor_tensor(
                            Pn[:], xs[:], 0.0, wvtiles.pop(idx)[:],
                            AO.is_lt, AO.mult)
                        p_next = t * 2 + bp
                        nc.gpsimd.tensor_tensor(
                            Pn[:], Pn[:], y2s[p_next][:, off:off + QL],
                            AO.add)
                        PYprev[bp][hq] = Pn
                    oeng = nc.sync if idx % 2 else nc.scalar
                    oeng.dma_start(outp[iA:iA + 2, :, off:off + QL],
                                   ottiles.pop(idx)[:])

                for idx in range(KPF):
                    issue_load(idx)
                for idx, (t, bp, hq) in enumerate(iters):
                    if idx + KPF < len(iters):
                        issue_load(idx + KPF)
                    p = (t - 1) * 2 + bp
                    off = QL * hq
                    xs = xstiles[idx]
                    py = y2s[p][:, off:off + QL] if t == 1 \
                        else PYprev[bp][hq][:]
                    nc.vector.affine_then_add(xs[:], xs[:], py,
                                              rsc2[:], nth2[:])
                    if t < 4:
                        wv2 = hf.tile([128, QL], F32, tag="wv", bufs=2)
                        nc.scalar.activation(wv2[:], xs[:], AF.Identity,
                                             bias=gw2[:],
                                             scale=1.0 - alpha2)
                        wvtiles[idx] = wv2
                    ot = hf.tile([128, QL], F8E4, tag="ot", bufs=2)
                    nc.scalar.activation(ot[:], xs[:], AF.Sign)
                    ottiles[idx] = ot
                    if idx > 0:
                        flush_tail(idx - 1)
                flush_tail(len(iters) - 1)

    nc.compile()
    return nc, names


def _sigmoid(x):
    return 1.0 / (1.0 + np.exp(-float(x)))


def prepare(x, conv1_w, bn1_gamma, bn1_beta, lif1_w, conv2_w, bn2_gamma,
            bn2_beta, lif2_w):
    x = np.ascontiguousarray(np.asarray(x, np.float32))
    conv1_w = np.asarray(conv1_w, np.float32)
    conv2_w = np.asarray(conv2_w, np.float32)

    a1 = _sigmoid(np.asarray(lif1_w).reshape(-1)[0])
    a2 = _sigmoid(np.asarray(lif2_w).reshape(-1)[0])

    key = (round(a1, 12), round(a2, 12))
    if key not in _prog_cache:
        _prog_cache[key] = _build(a1, a2)
    nc, names = _prog_cache[key]

    xh = x.astype(np.float16)
    xl = (x - xh.astype(np.float32)).astype(np.float16)
    xpad = np.zeros((T, B, C, 2, HP, HP), np.float16)
    xpad[:, :, :, 0, 1:57, 1:57] = xh
    xpad[:, :, :, 1, 1:57, 1:57] = xl
    xpad = np.ascontiguousarray(xpad.transpose(0, 1, 3, 2, 4, 5))

    xhf = xpad[:, :, 0].reshape(T, B, C, PP)
    xbs = np.zeros((T, B, 2, C, PP), np.float16)
    xbs[:, :, 0] = xhf
    xbs[:, :, 1, :, :PP - 1] = xhf[:, :, :, 1:]
    xcs = np.zeros((T, B, 2, C, PP), np.float16)
    xcs[:, :, 0, :, :PP - 2] = xhf[:, :, :, 2:]
    xcs[:, :, 1, :, :PP - HP - 2] = xhf[:, :, :, HP + 2:]

    w1h = conv1_w.astype(np.float16)
    w1l = (conv1_w - w1h.astype(np.float32)).astype(np.float16)

    def tapstack(wtop, wbot):
        out = np.zeros((128, 9 * 64), np.float16)
        for a in range(9):
            di, dj = a // 3, a % 3
            out[0:64, a * 64:(a + 1) * 64] = wtop[:, :, di, dj].T
            out[64:128, a * 64:(a + 1) * 64] = wbot[:, :, di, dj].T
        return out

    w1a_np = tapstack(w1h, w1h)
    w1q_np = np.zeros((128, 64), np.float16)
    w1q_np[0:64, :] = w1l[:, :, 0, 2].T
    w1q_np[64:128, :] = w1l[:, :, 1, 2].T
    w1p_np = np.zeros((128, 3 * 64), np.float16)
    w1s_np = np.zeros((128, 3 * 64), np.float16)
    for di in range(3):
        w1p_np[0:64, di * 64:(di + 1) * 64] = w1l[:, :, di, 0].T
        w1p_np[64:128, di * 64:(di + 1) * 64] = w1l[:, :, di, 1].T
        w1s_np[0:64, di * 64:(di + 1) * 64] = w1l[:, :, di, 2].T

    w0 = conv2_w.astype(E4)
    r1 = conv2_w - w0.astype(np.float32)
    w1t = (r1 * 16).astype(E4)
    r2 = r1 - w1t.astype(np.float32) / 16
    w2t = (r2 * 256).astype(E4)
    r3 = r2 - w2t.astype(np.float32) / 256
    w3t = (r3 * 4096).astype(E4)
    w2d1_np = np.zeros((128, 2, 9, 128), E4)
    w2d2_np = np.zeros((128, 2, 9, 128), E4)
    for a in range(9):
        di, dj = a // 3, a % 3
        for blk in range(2):
            sl = slice(64 * blk, 64 * blk + 64)
            w2d1_np[sl, 0, a, sl] = w0[:, :, di, dj].T
            w2d1_np[sl, 1, a, sl] = w1t[:, :, di, dj].T
            w2d2_np[sl, 0, a, sl] = w2t[:, :, di, dj].T
            w2d2_np[sl, 1, a, sl] = w3t[:, :, di, dj].T

    def dup(v):
        v = np.asarray(v, np.float32).reshape(64)
        return np.concatenate([v, v])

    cpar_np = np.zeros((128, 8), np.float32)
    cpar_np[:, 0] = dup(bn1_gamma)
    cpar_np[:, 1] = dup(bn1_beta)
    cpar_np[:, 2] = dup(bn2_gamma)
    cpar_np[:, 3] = dup(bn2_beta)
    cpar_np[:, 4] = 1.0 / (a1 * dup(bn1_gamma))
    cpar_np[:, 5] = 1.0 / (a2 * dup(bn2_gamma))
    cpar_np[:, 6] = 1.0 / dup(bn1_gamma)
    cpar_np[:, 7] = 1.0 / dup(bn2_gamma)

    kk, mm_ = np.meshgrid(np.arange(128), np.arange(128), indexing='ij')
    ident_np = (kk % 64 == mm_ % 64).astype(np.float32)

    in_maps = []
    for k in range(NCORES):
        xta_np = np.ascontiguousarray(
            xpad[:, 4 * k:4 * k + 4].reshape(NIMG, 2, 64, PP))
        xtb_np = np.ascontiguousarray(
            xbs[:, 4 * k:4 * k + 4].reshape(NIMG, 2, 64, PP))
        xtc_np = np.ascontiguousarray(
            xcs[:, 4 * k:4 * k + 4].reshape(NIMG, 2, 64, PP))
        xin_np = np.ascontiguousarray(
            x[:, 4 * k:4 * k + 4].reshape(NIMG, 64, PIX))
        in_maps.append({
            names['xta']: xta_np,
            names['xtb']: xtb_np,
            names['xtc']: xtc_np,
            names['w1q']: w1q_np,
            names['xin']: xin_np,
            names['w1a']: w1a_np,
            names['w1p']: w1p_np,
            names['w1s']: w1s_np,
            names['w2d1']: w2d1_np,
            names['w2d2']: w2d2_np,
            names['cpar']: cpar_np,
            names['ident']: ident_np,
        })

    return nc, names, in_maps


def kernel(**inputs):
    from concourse.bass_utils import run_bass_kernel_spmd
    nc, names, in_maps = prepare(**inputs)
    res = run_bass_kernel_spmd(nc, in_maps, core_ids=list(range(NCORES)))
    global LAST_RES, LAST_NAMES
    LAST_RES, LAST_NAMES = res, names
    out = np.empty((T, B, C, H, W), np.float32)
    for k in range(NCORES):
        o = res.results[k][names['outp']].astype(np.float32)
        o = (o >= -0.5).astype(np.float32)
        out[:, 4 * k:4 * k + 4] = o.reshape(T, BL, C, H, W)
    return out


if __name__ == "__main__":
    rng = np.random.default_rng(0)
    xs = rng.standard_normal((T, B, C, H, W)).astype(np.float32)
    w1 = (rng.standard_normal((64, 64, 3, 3)) * 0.05).astype(np.float32)
    w2 = (rng.standard_normal((64, 64, 3, 3)) * 0.05).astype(np.float32)
    o = kernel(x=xs, conv1_w=w1, bn1_gamma=np.ones(64, np.float32),
               bn1_beta=np.zeros(64, np.float32),
               lif1_w=np.zeros(1, np.float32), conv2_w=w2,
               bn2_gamma=np.ones(64, np.float32),
               bn2_beta=np.zeros(64, np.float32),
               lif2_w=np.zeros(1, np.float32))
    print("ran:", o.shape, float(o.mean()))


# revision 21
# speedup vs baseline: 1.3539x; 1.0136x over previous
import sys
sys.path.insert(0, '/opt/trn_rl_repo')

import numpy as np
import ml_dtypes

T, B, C, H, W = 4, 32, 64, 56, 56
NCORES = 8
BL = B // NCORES
NIMG = T * BL
HP = W + 2
PP = HP * HP
PIX = H * W
NCH = 7
CHW = 8 * W
NPAIR = 8
EPS = 1e-5
NG = float((T * B) * PIX)
QL = 14 * W
NQ = 4
E4 = ml_dtypes.float8_e4m3
E5 = ml_dtypes.float8_e5m2

_prog_cache = {}
NO_CC = False
TRACE = False
LAST_RES = None
LAST_NAMES = None


def _build(alpha1, alpha2):
    import concourse.mybir as mybir
    import concourse.tile as tile
    from concourse import bacc

    F32 = mybir.dt.float32
    F16 = mybir.dt.float16
    F8E4 = mybir.dt.float8e4
    F8E5 = mybir.dt.float8e5
    AO = mybir.AluOpType
    AF = mybir.ActivationFunctionType
    AX = mybir.AxisListType
    PM = mybir.MatmulPerfMode

    V1 = float(np.frombuffer(np.array([0x2C3C], np.uint16).tobytes(),
                             dtype=np.float16)[0])
    V2 = float(np.frombuffer(np.array([0x0C1C], np.uint16).tobytes(),
                             dtype=np.float16)[0])

    nc = bacc.Bacc(None, target_bir_lowering=False)
    names = {}

    with tile.TileContext(nc) as tc:
        with tc.tile_pool(name="dram", bufs=1, space="DRAM") as dram:
            xta = dram.tile([NIMG, 2, 64, PP], F16, kind="ExternalInput")
            xtb = dram.tile([NIMG, 2, 64, PP], F16, kind="ExternalInput")
            xtc = dram.tile([NIMG, 2, 64, PP], F16, kind="ExternalInput")
            xin = dram.tile([NIMG, 64, PIX], F32, kind="ExternalInput")
            w1a = dram.tile([128, 9 * 64], F16, kind="ExternalInput")
            w1p = dram.tile([128, 3 * 64], F16, kind="ExternalInput")
            w1s = dram.tile([128, 3 * 64], F16, kind="ExternalInput")
            w1q = dram.tile([128, 64], F16, kind="ExternalInput")
            w2d1 = dram.tile([128, 2, 9, 128], F8E4, kind="ExternalInput")
            w2d2 = dram.tile([128, 2, 9, 128], F8E4, kind="ExternalInput")
            cpar = dram.tile([128, 8], F32, kind="ExternalInput")
            ident = dram.tile([128, 128], F32, kind="ExternalInput")
            outp = dram.tile([NIMG, 64, PIX], F8E4, kind="ExternalOutput")
            names.update(xta=xta.name, xtb=xtb.name, xtc=xtc.name,
                         xin=xin.name,
                         w1a=w1a.name, w1p=w1p.name, w1s=w1s.name,
                         w1q=w1q.name,
                         w2d1=w2d1.name, w2d2=w2d2.name,
                         cpar=cpar.name, ident=ident.name,
                         outp=outp.name)

            with tc.tile_pool(name="dramw", bufs=1, space="DRAM") as dramw, \
                 tc.tile_pool(name="wsb", bufs=1) as wsb, \
                 tc.tile_pool(name="ys", bufs=8) as yspool, \
                 tc.tile_pool(name="plane", bufs=4) as plpool, \
                 tc.tile_pool(name="plb", bufs=2) as plbpool, \
                 tc.tile_pool(name="hfp", bufs=2) as hf, \
                 tc.tile_pool(name="tiny", bufs=8) as tiny, \
                 tc.tile_pool(name="ps", bufs=7, space="PSUM") as ps:

                w1as = wsb.tile([128, 9 * 64], F16, tag="w1a")
                nc.scalar.dma_start(w1as[:], w1a[:])
                w1ps = wsb.tile([128, 3 * 64], F16, tag="w1p")
                nc.scalar.dma_start(w1ps[:], w1p[:])
                w1ss = wsb.tile([128, 3 * 64], F16, tag="w1s")
                nc.scalar.dma_start(w1ss[:], w1s[:])
                w1qs = wsb.tile([128, 64], F16, tag="w1q")
                nc.scalar.dma_start(w1qs[:], w1q[:])
                w2d1s = wsb.tile([128, 2, 9, 128], F8E4, tag="w2d1")
                nc.scalar.dma_start(w2d1s[:], w2d1[:])
                w2d2s = wsb.tile([128, 2, 9, 128], F8E4, tag="w2d2")
                nc.scalar.dma_start(w2d2s[:], w2d2[:])
                cpars = wsb.tile([128, 8], F32, tag="cpar")
                nc.scalar.dma_start(cpars[:], cpar[:])
                idents = wsb.tile([128, 128], F32, tag="ident")
                nc.scalar.dma_start(idents[:], ident[:])
                sums1 = wsb.tile([128, 56], F32, tag="sums1")
                sums1q = wsb.tile([128, 56], F32, tag="sums1q")
                sums2 = wsb.tile([128, 56], F32, tag="sums2")
                sums2q = wsb.tile([128, 56], F32, tag="sums2q")
                scrv = wsb.tile([128, CHW], F32, tag="scrv")
                scra = wsb.tile([128, CHW], F32, tag="scra")
                sqwarm = tiny.tile([128, 1], F32, tag="t1")
                nc.vector.memset(sqwarm[:], 1.0)
                sqw2 = tiny.tile([128, 1], F32, tag="t1")
                nc.scalar.activation(sqw2[:], sqwarm[:], AF.Sqrt)

                def evac(pts, dst, sums_t, sumsq_t, col, on_act=False):
                    nc.scalar.activation(dst, pts[:], AF.Copy,
                                         accum_out=sums_t[:, col:col + 1])
                    if on_act:
                        nc.scalar.activation(
                            scra[:], dst, AF.Square,
                            accum_out=sumsq_t[:, col:col + 1])
                    else:
                        nc.vector.scalar_tensor_tensor(
                            scrv[:], dst, 1.0, dst, AO.bypass, AO.mult,
                            accum_out=sumsq_t[:, col:col + 1])

                def conv1_pair(plA, plB, pbA, pbB, pcA, pcB, dst_strip,
                               pcol):
                    plAr = plA.rearrange("p (r w) -> p r w", w=HP)
                    plBr = plB.rearrange("p (r w) -> p r w", w=HP)
                    pbAr = pbA.rearrange("p (r w) -> p r w", w=HP)
                    pbBr = pbB.rearrange("p (r w) -> p r w", w=HP)
                    pcAr = pcA.rearrange("p (r w) -> p r w", w=HP)
                    pcBr = pcB.rearrange("p (r w) -> p r w", w=HP)
                    for wave in (range(0, 4), range(4, 7)):
                        pts = {}
                        for cth in wave:
                            pts[cth] = ps.tile([128, CHW], F32, tag="ps",
                                               bufs=7, name=f"psum{cth}")
                        def hi_tap(a, start, stop):
                            di, dj = a // 3, a % 3
                            for cth in wave:
                                r0 = 8 * cth + di
                                for j, plr in enumerate((plAr, plBr)):
                                    out = pts[cth][64 * j:64 * (j + 1), :] \
                                        .rearrange("p (r w) -> p r w", r=8)
                                    nc.tensor.matmul(
                                        out, w1as[:, a * 64:(a + 1) * 64],
                                        plr[:, r0:r0 + 8, dj:dj + W],
                                        start=start, stop=stop,
                                        tile_position=(0, 64 * j),
                                        skip_group_check=True)

                        hi_tap(0, True, False)
                        for a in range(1, 5):
                            hi_tap(a, False, False)
                        for di in range(3):
                            for cth in wave:
                                r0 = 8 * cth + di
                                for j, pbr in enumerate((pbAr, pbBr)):
                                    out = pts[cth][64 * j:64 * (j + 1), :] \
                                        .rearrange("p (r w) -> p r w", r=8)
                                    nc.tensor.matmul(
                                        out, w1ps[:, di * 64:(di + 1) * 64],
                                        pbr[:, r0:r0 + 8, 0:W],
                                        start=False, stop=False,
                                        tile_position=(0, 64 * j),
                                        skip_group_check=True)
                        for cth in wave:
                            r0 = 8 * cth
                            for j, pcr in enumerate((pcAr, pcBr)):
                                out = pts[cth][64 * j:64 * (j + 1), :] \
                                    .rearrange("p (r w) -> p r w", r=8)
                                nc.tensor.matmul(
                                    out, w1qs[:, 0:64],
                                    pcr[:, r0:r0 + 8, 0:W],
                                    start=False, stop=False,
                                    tile_position=(0, 64 * j),
                                    skip_group_check=True)
                        for cth in wave:
                            r0 = 8 * cth + 2
                            for j, plr in enumerate((plAr, plBr)):
                                out = pts[cth][64 * j:64 * (j + 1), :] \
                                    .rearrange("p (r w) -> p r w", r=8)
                                nc.tensor.matmul(
                                    out, w1ss[0:64, 2 * 64:3 * 64],
                                    plr[0:64, r0:r0 + 8, 2:2 + W],
                                    start=False, stop=False,
                                    tile_position=(0, 64 * j),
                                    skip_group_check=True)
                        for a in range(5, 9):
                            hi_tap(a, False, a == 8)
                        for cth in wave:
                            evac(pts[cth],
                                 dst_strip[:, CHW * cth:CHW * (cth + 1)],
                                 sums1, sums1q, pcol * 7 + cth,
                                 on_act=(cth % 2 == 1))

                def conv2_pair(sp1, sp2, dst_strip, pcol):
                    p14 = sp1.rearrange("p (r w) k -> p k r w", w=HP)
                    p24 = sp2.rearrange("p (r w) k -> p k r w", w=HP)
                    for wave in (range(0, 4), range(4, 7)):
                        pts = {}
                        for cth in wave:
                            pts[cth] = ps.tile([128, CHW], F32, tag="ps",
                                               bufs=7, name=f"psum{cth}")
                        for pli, (pl4, wd) in enumerate(
                                ((p14, w2d1s), (p24, w2d2s))):
                            for a in range(9):
                                di, dj = a // 3, a % 3
                                for cth in wave:
                                    r0 = 8 * cth + di
                                    out = pts[cth][:].rearrange(
                                        "p (r w) -> p r w", r=8)
                                    nc.tensor.matmul(
                                        out, wd[:, :, a, :],
                                        pl4[:, :, r0:r0 + 8, dj:dj + W],
                                        start=(pli == 0 and a == 0),
                                        stop=(pli == 1 and a == 8),
                                        perf_mode=PM.DoubleRow,
                                        skip_group_check=True)
                        for cth in wave:
                            evac(pts[cth],
                                 dst_strip[:, CHW * cth:CHW * (cth + 1)],
                                 sums2, sums2q, pcol * 7 + cth,
                                 on_act=(cth % 2 == 1))

                y1s = []
                for p in range(NPAIR):
                    tt_, bp = p // 2, p % 2
                    iA = tt_ * 4 + bp * 2
                    tas, pbs, pcs = [], [], []
                    for j in range(2):
                        i = iA + j
                        ta = plpool.tile([128, PP], F16, tag="pl")
                        tar = ta.rearrange("p (r w) -> p r w", w=HP)
                        xtr = xta[i].rearrange("a c (r w) -> a c r w", w=HP)
                        nc.sync.dma_start(tar[:, 0:29, :],
                                          xtr[:, :, 0:29, :])
                        nc.gpsimd.dma_start(tar[:, 29:HP, :],
                                            xtr[:, :, 29:HP, :])
                        tas.append(ta)
                        pb = plbpool.tile([128, PP], F16, tag="plb",
                                          bufs=4)
                        nc.scalar.dma_start(pb[:], xtb[i])
                        pbs.append(pb)
                        pc = plbpool.tile([128, PP], F16, tag="plb",
                                          bufs=4)
                        nc.sync.dma_start(pc[:], xtc[i])
                        pcs.append(pc)
                    strip = yspool.tile([128, PIX], F32, tag=f"ys{p}",
                                        bufs=1)
                    y1s.append(strip)
                    conv1_pair(tas[0], tas[1], pbs[0], pbs[1], pcs[0],
                               pcs[1], strip, p)

                cc1i = dramw.tile([128, 2], F32)
                cc1o = dramw.tile([128, 2], F32, addr_space="Shared")
                acc1 = tiny.tile([128, 2], F32, tag="acc")
                nc.vector.tensor_reduce(acc1[:, 0:1], sums1[:], AX.X, AO.add)
                nc.vector.tensor_reduce(acc1[:, 1:2], sums1q[:], AX.X, AO.add)
                nc.sync.dma_start(cc1i[:], acc1[:])
                if NO_CC:
                    nc.sync.dma_start(cc1o[:], cc1i[:])
                else:
                    nc.gpsimd.collective_compute(
                        "AllReduce", AO.add, ins=[cc1i[:]], outs=[cc1o[:]],
                        replica_groups=[list(range(NCORES))])
                g1 = tiny.tile([128, 2], F32, tag="acc")
                nc.sync.dma_start(g1[:], cc1o[:])

                def stats_block(g, gamma, beta, rga, rgam, alpha):
                    totp = ps.tile([128, CHW], F32, tag="pstot", bufs=1,
                                   name="pstot")
                    nc.tensor.matmul(totp[:, 0:2], idents[:], g[:],
                                     start=True, stop=True,
                                     skip_group_check=True)
                    tot = totp[:, 0:2]
                    mm = tiny.tile([128, 2], F32, tag="acc")
                    nc.vector.tensor_scalar(mm[:], tot, 1.0 / NG, None,
                                            AO.mult)
                    mean = mm[:, 0:1]
                    m2 = tiny.tile([128, 1], F32, tag="t1")
                    nc.vector.tensor_tensor(m2[:], mean, mean, AO.mult)
                    var = tiny.tile([128, 1], F32, tag="t1")
                    nc.vector.tensor_tensor(var[:], mm[:, 1:2], m2[:],
                                            AO.subtract)
                    epst = tiny.tile([128, 1], F32, tag="t1")
                    nc.vector.memset(epst[:], EPS)
                    std = tiny.tile([128, 1], F32, tag="t1")
                    nc.scalar.activation(std[:], var[:], AF.Sqrt, bias=epst[:])
                    rstd = tiny.tile([128, 1], F32, tag="t1")
                    nc.vector.reciprocal(rstd[:], std[:])
                    rscv = tiny.tile([128, 1], F32, tag="t1")
                    nc.vector.tensor_tensor(rscv[:], std[:], rgam, AO.mult)
                    sc = tiny.tile([128, 1], F32, tag="t1")
                    nc.vector.tensor_tensor(sc[:], gamma, rstd[:], AO.mult)
                    nmsc = tiny.tile([128, 1], F32, tag="t1")
                    nc.vector.scalar_tensor_tensor(nmsc[:], mean, -1.0, sc[:],
                                                   AO.mult, AO.mult)
                    bi = tiny.tile([128, 1], F32, tag="t1")
                    nc.vector.tensor_tensor(bi[:], beta, nmsc[:], AO.add)
                    stdrg = tiny.tile([128, 1], F32, tag="t1")
                    nc.vector.tensor_tensor(stdrg[:], std[:], rga, AO.mult)
                    nbst = tiny.tile([128, 1], F32, tag="t1")
                    nc.vector.scalar_tensor_tensor(nbst[:], bi[:], -alpha,
                                                   stdrg[:], AO.mult, AO.mult)
                    th = tiny.tile([128, 1], F32, tag="t1")
                    nc.vector.tensor_tensor(th[:], stdrg[:], nbst[:], AO.add)
                    bstd = tiny.tile([128, 1], F32, tag="t1")
                    nc.vector.tensor_tensor(bstd[:], bi[:], std[:], AO.mult)
                    gamv = tiny.tile([128, 1], F32, tag="t1")
                    nc.vector.tensor_tensor(gamv[:], bstd[:], rgam, AO.mult)
                    gmw = tiny.tile([128, 1], F32, tag="t1")
                    nc.vector.tensor_scalar(gmw[:], gamv[:], 1.0 - alpha, None,
                                            AO.mult)
                    return th, gamv, rscv, gmw

                th1, gm1, _rsc1, gmw1 = stats_block(
                    g1, cpars[:, 0:1], cpars[:, 1:2], cpars[:, 4:5],
                    cpars[:, 6:7], alpha1)

                y2s = [None] * NPAIR
                for bp in range(2):
                    Pprev = [None] * NQ
                    for t in range(1, 5):
                        p = (t - 1) * 2 + bp
                        sp1 = plpool.tile([128, PP, 2], F8E5, tag="pl")
                        sp2 = plpool.tile([128, PP, 2], F8E5, tag="pl")
                        w1v = sp1.rearrange("p (r w) k -> p r w k", w=HP)
                        w2v = sp2.rearrange("p (r w) k -> p r w k", w=HP)
                        for sp4 in (w1v, w2v):
                            nc.gpsimd.memset(sp4[:, 0:1, :, :], 0.0)
                            nc.gpsimd.memset(sp4[:, HP - 1:HP, :, :], 0.0)
                            nc.gpsimd.memset(sp4[:, 1:HP - 1, 0:1, :], 0.0)
                            nc.gpsimd.memset(sp4[:, 1:HP - 1,
                                                 HP - 1:HP, :], 0.0)
                        for hq in range(NQ):
                            off = QL * hq
                            ysl = y1s[p][:, off:off + QL]
                            if t == 1:
                                qa = ysl
                            else:
                                q = hf.tile([128, QL], F32, tag="q2", bufs=4)
                                if hq % 2 == 0:
                                    nc.vector.affine_then_add(
                                        q[:], Pprev[hq][:], ysl, 1.0, 0.0)
                                else:
                                    nc.gpsimd.tensor_tensor(
                                        q[:], ysl, Pprev[hq][:], AO.add)
                                qa = q[:]
                            qa3 = qa.rearrange("p (r w) -> p r w", w=W)
                            r0, r1_ = 1 + 14 * hq, 15 + 14 * hq
                            d1 = w1v[:, r0:r1_, 1:1 + W, :].bitcast(F16)
                            d2 = w2v[:, r0:r1_, 1:1 + W, :].bitcast(F16)
                            seng = nc.vector if hq % 2 == 0 else nc.gpsimd
                            nc.vector.tensor_scalar(d1, qa3, th1[:],
                                                    V1, AO.is_ge, AO.mult)
                            seng.tensor_scalar(d2, qa3, th1[:],
                                               V2, AO.is_ge, AO.mult)
                            if t < 4:
                                wv = hf.tile([128, QL], F32, tag="wv",
                                             bufs=2)
                                nc.scalar.activation(wv[:], qa, AF.Identity,
                                                     bias=gmw1[:],
                                                     scale=1.0 - alpha1)
                                Pn = hf.tile([128, QL], F32, tag="pp",
                                             bufs=4)
                                nc.vector.scalar_tensor_tensor(
                                    Pn[:], qa, th1[:], wv[:],
                                    AO.is_lt, AO.mult)
                                Pprev[hq] = Pn
                        strip2 = yspool.tile([128, PIX], F32, tag=f"ys{p}",
                                             bufs=1)
                        y2s[p] = strip2
                        conv2_pair(sp1, sp2, strip2, p)

                cc2i = dramw.tile([128, 2], F32)
                cc2o = dramw.tile([128, 2], F32, addr_space="Shared")
                acc2 = tiny.tile([128, 2], F32, tag="acc")
                nc.vector.tensor_reduce(acc2[:, 0:1], sums2[:], AX.X, AO.add)
                nc.vector.tensor_reduce(acc2[:, 1:2], sums2q[:], AX.X, AO.add)
                nc.sync.dma_start(cc2i[:], acc2[:])
                if NO_CC:
                    nc.sync.dma_start(cc2o[:], cc2i[:])
                else:
                    nc.gpsimd.collective_compute(
                        "AllReduce", AO.add, ins=[cc2i[:]], outs=[cc2o[:]],
                        replica_groups=[list(range(NCORES))])
                g2 = tiny.tile([128, 2], F32, tag="acc")
                nc.sync.dma_start(g2[:], cc2o[:])
                th2, gm2, rsc2, gmw2 = stats_block(
                    g2, cpars[:, 2:3], cpars[:, 3:4], cpars[:, 5:6],
                    cpars[:, 7:8], alpha2)
                nth2 = tiny.tile([128, 1], F32, tag="t1")
                nc.vector.tensor_scalar(nth2[:], th2[:], -1.0, None, AO.mult)
                gw2 = tiny.tile([128, 1], F32, tag="t1")
                nc.vector.scalar_tensor_tensor(gw2[:], th2[:], 1.0 - alpha2,
                                               gmw2[:], AO.mult, AO.add)

                PYprev = {0: [None] * NQ, 1: [None] * NQ}
                iters = [(t, bp, hq) for t in range(1, 5)
                         for bp in range(2) for hq in range(NQ)]
                KPF = 2
                xstiles = {}

                def issue_load(idx):
                    t, bp, hq = iters[idx]
                    iA = (t - 1) * 4 + bp * 2
                    off = QL * hq
                    xs = hf.tile([128, QL], F32, tag="xs", bufs=4)
                    nc.sync.dma_start(xs[:],
                                      xin[iA:iA + 2, :, off:off + QL])
                    xstiles[idx] = xs

                def flush_tail(idx):
                    t, bp, hq = iters[idx]
                    iA = (t - 1) * 4 + bp * 2
                    off = QL * hq
                    xs = xstiles.pop(idx)
                    if t < 4:
                        wv2 = hf.tile([128, QL], F32, tag="wv", bufs=2)
                        nc.scalar.activation(wv2[:], xs[:], AF.Identity,
                                             bias=gw2[:],
                                             scale=1.0 - alpha2)
                    ot = hf.tile([128, QL], F8E4, tag="ot", bufs=2)
                    nc.scalar.activation(ot[:], xs[:], AF.Sign)
                    if t < 4:
                        ptag = ("pp", 4) if bp == 0 else ("q2", 4)
                        Pn = hf.tile([128, QL], F32, tag=ptag[0],
                                     bufs=ptag[1])
                        nc.vector.scalar_tensor_tensor(
                            Pn[:], xs[:], 0.0, wv2[:],
                            AO.is_lt, AO.mult)
                        p_next = t * 2 + bp
                        nc.gpsimd.tensor_tensor(
                            Pn[:], Pn[:], y2s[p_next][:, off:off + QL],
                            AO.add)
                        PYprev[bp][hq] = Pn
                    oeng = nc.sync if idx % 2 else nc.scalar
                    oeng.dma_start(outp[iA:iA + 2, :, off:off + QL],
                                   ot[:])

                for idx in range(KPF):
                    issue_load(idx)
                for idx, (t, bp, hq) in enumerate(iters):
                    if idx + KPF < len(iters):
                        issue_load(idx + KPF)
                    p = (t - 1) * 2 + bp
                    off = QL * hq
                    xs = xstiles[idx]
                    py = y2s[p][:, off:off + QL] if t == 1 \
                        else PYprev[bp][hq][:]
                    nc.vector.affine_then_add(xs[:], xs[:], py,
                                              rsc2[:], nth2[:])
                    if idx > 0:
                        flush_tail(idx - 1)
                flush_tail(len(iters) - 1)

    nc.compile()
    return nc, names


def _sigmoid(x):
    return 1.0 / (1.0 + np.exp(-float(x)))


def prepare(x, conv1_w, bn1_gamma, bn1_beta, lif1_w, conv2_w, bn2_gamma,
            bn2_beta, lif2_w):
    x = np.ascontiguousarray(np.asarray(x, np.float32))
    conv1_w = np.asarray(conv1_w, np.float32)
    conv2_w = np.asarray(conv2_w, np.float32)

    a1 = _sigmoid(np.asarray(lif1_w).reshape(-1)[0])
    a2 = _sigmoid(np.asarray(lif2_w).reshape(-1)[0])

    key = (round(a1, 12), round(a2, 12))
    if key not in _prog_cache:
        _prog_cache[key] = _build(a1, a2)
    nc, names = _prog_cache[key]

    xh = x.astype(np.float16)
    xl = (x - xh.astype(np.float32)).astype(np.float16)
    xpad = np.zeros((T, B, C, 2, HP, HP), np.float16)
    xpad[:, :, :, 0, 1:57, 1:57] = xh
    xpad[:, :, :, 1, 1:57, 1:57] = xl
    xpad = np.ascontiguousarray(xpad.transpose(0, 1, 3, 2, 4, 5))

    xhf = xpad[:, :, 0].reshape(T, B, C, PP)
    xbs = np.zeros((T, B, 2, C, PP), np.float16)
    xbs[:, :, 0] = xhf
    xbs[:, :, 1, :, :PP - 1] = xhf[:, :, :, 1:]
    xcs = np.zeros((T, B, 2, C, PP), np.float16)
    xcs[:, :, 0, :, :PP - 2] = xhf[:, :, :, 2:]
    xcs[:, :, 1, :, :PP - HP - 2] = xhf[:, :, :, HP + 2:]

    w1h = conv1_w.astype(np.float16)
    w1l = (conv1_w - w1h.astype(np.float32)).astype(np.float16)

    def tapstack(wtop, wbot):
        out = np.zeros((128, 9 * 64), np.float16)
        for a in range(9):
            di, dj = a // 3, a % 3
            out[0:64, a * 64:(a + 1) * 64] = wtop[:, :, di, dj].T
            out[64:128, a * 64:(a + 1) * 64] = wbot[:, :, di, dj].T
        return out

    w1a_np = tapstack(w1h, w1h)
    w1q_np = np.zeros((128, 64), np.float16)
    w1q_np[0:64, :] = w1l[:, :, 0, 2].T
    w1q_np[64:128, :] = w1l[:, :, 1, 2].T
    w1p_np = np.zeros((128, 3 * 64), np.float16)
    w1s_np = np.zeros((128, 3 * 64), np.float16)
    for di in range(3):
        w1p_np[0:64, di * 64:(di + 1) * 64] = w1l[:, :, di, 0].T
        w1p_np[64:128, di * 64:(di + 1) * 64] = w1l[:, :, di, 1].T
        w1s_np[0:64, di * 64:(di + 1) * 64] = w1l[:, :, di, 2].T

    w0 = conv2_w.astype(E4)
    r1 = conv2_w - w0.astype(np.float32)
    w1t = (r1 * 16).astype(E4)
    r2 = r1 - w1t.astype(np.float32) / 16
    w2t = (r2 * 256).astype(E4)
    r3 = r2 - w2t.astype(np.float32) / 256
    w3t = (r3 * 4096).astype(E4)
    w2d1_np = np.zeros((128, 2, 9, 128), E4)
    w2d2_np = np.zeros((128, 2, 9, 128), E4)
    for a in range(9):
        di, dj = a // 3, a % 3
        for blk in range(2):
            sl = slice(64 * blk, 64 * blk + 64)
            w2d1_np[sl, 0, a, sl] = w0[:, :, di, dj].T
            w2d1_np[sl, 1, a, sl] = w1t[:, :, di, dj].T
            w2d2_np[sl, 0, a, sl] = w2t[:, :, di, dj].T
            w2d2_np[sl, 1, a, sl] = w3t[:, :, di, dj].T

    def dup(v):
        v = np.asarray(v, np.float32).reshape(64)
        return np.concatenate([v, v])

    cpar_np = np.zeros((128, 8), np.float32)
    cpar_np[:, 0] = dup(bn1_gamma)
    cpar_np[:, 1] = dup(bn1_beta)
    cpar_np[:, 2] = dup(bn2_gamma)
    cpar_np[:, 3] = dup(bn2_beta)
    cpar_np[:, 4] = 1.0 / (a1 * dup(bn1_gamma))
    cpar_np[:, 5] = 1.0 / (a2 * dup(bn2_gamma))
    cpar_np[:, 6] = 1.0 / dup(bn1_gamma)
    cpar_np[:, 7] = 1.0 / dup(bn2_gamma)

    kk, mm_ = np.meshgrid(np.arange(128), np.arange(128), indexing='ij')
    ident_np = (kk % 64 == mm_ % 64).astype(np.float32)

    in_maps = []
    for k in range(NCORES):
        xta_np = np.ascontiguousarray(
            xpad[:, 4 * k:4 * k + 4].reshape(NIMG, 2, 64, PP))
        xtb_np = np.ascontiguousarray(
            xbs[:, 4 * k:4 * k + 4].reshape(NIMG, 2, 64, PP))
        xtc_np = np.ascontiguousarray(
            xcs[:, 4 * k:4 * k + 4].reshape(NIMG, 2, 64, PP))
        xin_np = np.ascontiguousarray(
            x[:, 4 * k:4 * k + 4].reshape(NIMG, 64, PIX))
        in_maps.append({
            names['xta']: xta_np,
            names['xtb']: xtb_np,
            names['xtc']: xtc_np,
            names['w1q']: w1q_np,
            names['xin']: xin_np,
            names['w1a']: w1a_np,
            names['w1p']: w1p_np,
            names['w1s']: w1s_np,
            names['w2d1']: w2d1_np,
            names['w2d2']: w2d2_np,
            names['cpar']: cpar_np,
            names['ident']: ident_np,
        })

    return nc, names, in_maps


def kernel(**inputs):
    from concourse.bass_utils import run_bass_kernel_spmd
    nc, names, in_maps = prepare(**inputs)
    res = run_bass_kernel_spmd(nc, in_maps, core_ids=list(range(NCORES)))
    global LAST_RES, LAST_NAMES
    LAST_RES, LAST_NAMES = res, names
    out = np.empty((T, B, C, H, W), np.float32)
    for k in range(NCORES):
        o = res.results[k][names['outp']].astype(np.float32)
        o = (o >= -0.5).astype(np.float32)
        out[:, 4 * k:4 * k + 4] = o.reshape(T, BL, C, H, W)
    return out


if __name__ == "__main__":
    rng = np.random.default_rng(0)
    xs = rng.standard_normal((T, B, C, H, W)).astype(np.float32)
    w1 = (rng.standard_normal((64, 64, 3, 3)) * 0.05).astype(np.float32)
    w2 = (rng.standard_normal((64, 64, 3, 3)) * 0.05).astype(np.float32)
    o = kernel(x=xs, conv1_w=w1, bn1_gamma=np.ones(64, np.float32),
               bn1_beta=np.zeros(64, np.float32),
               lif1_w=np.zeros(1, np.float32), conv2_w=w2,
               bn2_gamma=np.ones(64, np.float32),
               bn2_beta=np.zeros(64, np.float32),
               lif2_w=np.zeros(1, np.float32))
    print("ran:", o.shape, float(o.mean()))
